# revision 22
# baseline (speedup 1.0000x reference)
"""Depthwise-separable conv block (dw3x3 + BN + ReLU + pw1x1 + BN + ReLU)
for Trainium2, data-parallel over batch across 8 NeuronCores with sync-BN
via two tiny AllReduces.

Key design points:
  - Depthwise conv = 9 PSUM-accumulated diagonal matmuls per pixel chunk
    (float32r => full PE rate at N>=256).
  - BN in training mode absorbs the conv biases (dw_b, pw_b shift the mean
    which BN subtracts), so they are dropped entirely.
  - BN1 folds to per-channel affine applied by one ScalarE op
    h = relu(a1*t + c1); BN2 folds into the PSUM eviction of the pointwise
    matmul: out = relu(a2*y + c2).
  - BN2 stats need a first pointwise pass (discarded into bn_stats); the
    final pass recomputes y after AllReduce-2 (PE is otherwise idle then).
"""

import numpy as np

import concourse.bass as bass
import concourse.tile as tile
import concourse.mybir as mybir
from concourse import bass_utils

N_CORES = 8
C = 128          # input channels (= SBUF partitions)
O = 256          # output channels
H = W = 112
HP = WP = 114    # zero-padded input
IMG_PER_CORE = 2
PIX_PER_IMG = H * W                 # 12544
PIX_TOTAL = IMG_PER_CORE * PIX_PER_IMG  # 25088
EPS = 1e-5

F32 = mybir.dt.float32
F32R = mybir.dt.float32r

DMA_ROWS = 16    # output rows per input DMA chunk (loads DMA_ROWS+2 rows)
SUB_ROWS = 4     # output rows per conv matmul chunk (N = 448)
PW_CHUNK = 512   # pixels per pointwise matmul (one PSUM bank of f32)

def _legalize_waits(nc):
    """Split multi-wait instructions: this walrus build's codegen accepts at
    most ONE sync wait per ISA instruction, while Tile's sem-assignment
    freely attaches several. Move all but one semaphore wait onto freshly
    inserted NoOps on the same engine directly before the instruction
    (waits are AND-semantics, so order is irrelevant)."""
    cnt = 0
    for bb in nc.main_func.blocks:
        new = []
        for ins in bb.instructions:
            si = ins.sync_info
            if si is not None and len(si.on_wait) > 1:
                sem_waits = [w for w in si.on_wait if w.sync_type == "semaphore"]
                other = [w for w in si.on_wait if w.sync_type != "semaphore"]
                keep = other + sem_waits[-1:] if not other else other
                move = sem_waits[:-1] if not other else sem_waits
                if len(keep) <= 1 and move:
                    for w in move:
                        cnt += 1
                        nop = mybir.InstNoOp(name=f"I-waitnop{cnt}", ins=[], outs=[])
                        nop.engine = ins.engine
                        nop.sync_info = mybir.SyncInfo(on_wait=[w], on_update=[])
                        new.append(nop)
                    ins.sync_info = mybir.SyncInfo(
                        on_wait=keep, on_update=list(si.on_update)
                    )
            new.append(ins)
        try:
            bb.instructions[:] = new
        except TypeError:
            bb.instructions = new
    return cnt


def _build_program(collectives=True):
    nc = bass.Bass(
        "TRN2",
        target_bir_lowering=False,
        debug=False,
        num_devices=N_CORES if collectives else 1,
    )

    # float32r = same 4-byte layout as f32 but lets the PE run matmuls at
    # full rate (fp32 proper is 4 cycles/row); the BIR verifier requires the
    # whole producer chain of a matmul operand to carry the f32r dtype.
    #
    # All constants are packed into ONE tensor so they arrive via one DMA on
    # one DMA lane: Matmult instructions only support a single sync wait, so
    # the first matmul cannot wait on separate weight+data DMA lanes.
    # Layout per channel row: [dwdiag 9*128 | pwT 256 | g1 b1 | g2h0 g2h1 b2h0 b2h1]
    NCONST = 9 * C + O + 2 + 4
    xp = nc.dram_tensor("xp", (IMG_PER_CORE, C, HP, WP), F32R, kind="ExternalInput").ap()
    cst = nc.dram_tensor("cst", (C, NCONST), F32R, kind="ExternalInput").ap()
    y = nc.dram_tensor("y", (IMG_PER_CORE, O, H, W), F32, kind="ExternalOutput").ap()
    y_r = y.rearrange("n c h w -> n c (h w)")

    n_conv_chunks = IMG_PER_CORE * (H // SUB_ROWS)              # 56
    n_pw_chunks = IMG_PER_CORE * ((PIX_PER_IMG + PW_CHUNK - 1) // PW_CHUNK)  # 50

    with tile.TileContext(nc) as tc:
        with (
            tc.tile_pool(name="consts", bufs=1) as consts,
            tc.tile_pool(name="big", bufs=1) as big,
            tc.tile_pool(name="xin", bufs=3) as xin,
            tc.tile_pool(name="stats", bufs=1) as stats,
            tc.tile_pool(name="yout", bufs=4) as yout,
            tc.tile_pool(name="psum", bufs=4, space="PSUM") as psum,
            tc.tile_pool(name="dram", bufs=1, space="DRAM") as dram,
        ):
            # ---- constants (single DMA; see NCONST layout note above) -----
            cst_sb = consts.tile([C, NCONST], F32R)
            nc.gpsimd.dma_start(out=cst_sb, in_=cst)
            dwdiag_sb = cst_sb[:, 0 : 9 * C].rearrange("p (t c) -> p t c", t=9)
            pwt_sb = cst_sb[:, 9 * C : 9 * C + O]
            bn1gb_sb = cst_sb[:, 9 * C + O : 9 * C + O + 2].bitcast(F32)
            bn2gb_sb = cst_sb[:, 9 * C + O + 2 : 9 * C + O + 6].bitcast(F32)
            eps_sb = consts.tile([C, 1], F32)
            nc.vector.memset(eps_sb, EPS)

            # Matmult instructions only support ONE sync wait in codegen, but
            # real matmuls here naturally depend on two producers (weight/data
            # DMA + PSUM-slot release, or ACT h-apply + PSUM release). Tiny
            # "observer" matmuls into a private write-only PSUM tile advance
            # PE's vector clock for one producer so each real matmul needs at
            # most one wait. obs_ps is reused write-only (same-engine WAW, no
            # semaphores).
            obs_ps = psum.tile([8, 8], F32, tag="obs")

            def pe_observe(src_ap):
                nc.tensor.matmul(
                    obs_ps, src_ap[:, 0:8], src_ap[:, 0:8], start=True, stop=True
                )

            pe_observe(cst_sb)

            # depthwise-conv output, SBUF-resident for the whole kernel
            t_sb = big.tile([C, PIX_TOTAL], F32)

            stats1 = stats.tile([C, n_conv_chunks, 6], F32)
            stats2 = stats.tile([C, 2, n_pw_chunks, 6], F32)

            # ---- stage A: depthwise conv + BN1 partial stats --------------
            ci = 0
            for n in range(IMG_PER_CORE):
                for rblk in range(0, H, DMA_ROWS):
                    x_t = xin.tile([C, DMA_ROWS + 2, WP], F32R, tag="x")
                    nc.gpsimd.dma_start(
                        out=x_t, in_=xp[n, :, rblk : rblk + DMA_ROWS + 2, :]
                    )
                    pe_observe(x_t[:, 0, :])
                    for sr in range(0, DMA_ROWS, SUB_ROWS):
                        pt = psum.tile([C, SUB_ROWS * W], F32, tag="ps")
                        for t9 in range(9):
                            di, dj = divmod(t9, 3)
                            rhs = x_t[:, sr + di : sr + di + SUB_ROWS, dj : dj + W]
                            nc.tensor.matmul(
                                pt,
                                dwdiag_sb[:, t9, :],
                                rhs,
                                start=(t9 == 0),
                                stop=(t9 == 8),
                            )
                        off = n * PIX_PER_IMG + (rblk + sr) * W
                        tsl = t_sb[:, off : off + SUB_ROWS * W]
                        nc.scalar.copy(out=tsl.bitcast(F32R), in_=pt)
                        nc.vector.bn_stats(out=stats1[:, ci : ci + 1, :], in_=tsl)
                        ci += 1

            # ---- BN1: aggregate, AllReduce, fold --------------------------
            mv1 = stats.tile([C, 2], F32)
            nc.vector.bn_aggr(out=mv1, in_=stats1)
            ar1_sb = stats.tile([C, 2], F32)
            msq1 = stats.tile([C, 1], F32)
            nc.vector.tensor_copy(out=ar1_sb[:, 0:1], in_=mv1[:, 0:1])
            nc.vector.tensor_mul(out=msq1, in0=mv1[:, 0:1], in1=mv1[:, 0:1])
            nc.vector.tensor_add(out=ar1_sb[:, 1:2], in0=mv1[:, 1:2], in1=msq1)

            ar1_in = dram.tile([C, 2], F32)
            ar1_out = dram.tile([C, 2], F32)
            nc.gpsimd.dma_start(out=ar1_in, in_=ar1_sb)
            if collectives:
                nc.gpsimd.collective_compute(
                    "AllReduce",
                    mybir.AluOpType.add,
                    replica_groups=[list(range(N_CORES))],
                    ins=[ar1_in.opt()],
                    outs=[ar1_out.opt()],
                )
            else:  # timing-sim variant: skip the collective
                nc.gpsimd.dma_start(out=ar1_out, in_=ar1_in)
            g1 = stats.tile([C, 2], F32)
            nc.gpsimd.dma_start(out=g1, in_=ar1_out)

            gm1 = stats.tile([C, 1], F32)
            gex1 = stats.tile([C, 1], F32)
            gvar1 = stats.tile([C, 1], F32)
            a1 = stats.tile([C, 1], F32)
            c1 = stats.tile([C, 1], F32)
            nc.scalar.mul(out=gm1, in_=g1[:, 0:1], mul=1.0 / N_CORES)
            nc.scalar.mul(out=gex1, in_=g1[:, 1:2], mul=1.0 / N_CORES)
            nc.vector.tensor_mul(out=gvar1, in0=gm1, in1=gm1)
            nc.vector.tensor_sub(out=gvar1, in0=gex1, in1=gvar1)
            nc.scalar.activation(
                out=gvar1, in_=gvar1,
                func=mybir.ActivationFunctionType.Sqrt,
                bias=eps_sb, scale=1.0,
            )
            nc.vector.reciprocal(out=a1, in_=gvar1)
            nc.vector.tensor_mul(out=a1, in0=a1, in1=bn1gb_sb[:, 0:1])
            nc.vector.tensor_mul(out=c1, in0=gm1, in1=a1)
            nc.vector.tensor_sub(out=c1, in0=bn1gb_sb[:, 1:2], in1=c1)

            # ---- stage B: h = relu(a1*t + c1); pw pass 1 -> BN2 stats -----
            cj = 0
            for n in range(IMG_PER_CORE):
                for p0 in range(0, PIX_PER_IMG, PW_CHUNK):
                    sz = min(PW_CHUNK, PIX_PER_IMG - p0)
                    off = n * PIX_PER_IMG + p0
                    hsl = t_sb[:, off : off + sz]
                    nc.scalar.activation(
                        out=hsl.bitcast(F32R), in_=hsl,
                        func=mybir.ActivationFunctionType.Relu,
                        bias=c1, scale=a1,
                    )
                    pe_observe(hsl.bitcast(F32R))
                    for hf in range(2):
                        py = psum.tile([C, PW_CHUNK], F32, tag="ps")
                        nc.tensor.matmul(
                            py[:, :sz],
                            pwt_sb[:, hf * 128 : (hf + 1) * 128],
                            hsl.bitcast(F32R),
                            start=True, stop=True,
                        )
                        nc.vector.bn_stats(
                            out=stats2[:, hf, cj : cj + 1, :], in_=py[:, :sz]
                        )
                    cj += 1

            # ---- BN2: aggregate, AllReduce, fold --------------------------
            mv2 = stats.tile([C, 2, 2], F32)
            ar2_sb = stats.tile([C, 4], F32)
            msq2 = stats.tile([C, 1], F32)
            for hf in range(2):
                nc.vector.bn_aggr(out=mv2[:, hf, :], in_=stats2[:, hf, :, :])
                nc.vector.tensor_copy(
                    out=ar2_sb[:, 2 * hf : 2 * hf + 1], in_=mv2[:, hf, 0:1]
                )
                nc.vector.tensor_mul(out=msq2, in0=mv2[:, hf, 0:1], in1=mv2[:, hf, 0:1])
                nc.vector.tensor_add(
                    out=ar2_sb[:, 2 * hf + 1 : 2 * hf + 2],
                    in0=mv2[:, hf, 1:2], in1=msq2,
                )

            ar2_in = dram.tile([C, 4], F32)
            ar2_out = dram.tile([C, 4], F32)
            nc.gpsimd.dma_start(out=ar2_in, in_=ar2_sb)
            if collectives:
                nc.gpsimd.collective_compute(
                    "AllReduce",
                    mybir.AluOpType.add,
                    replica_groups=[list(range(N_CORES))],
                    ins=[ar2_in.opt()],
                    outs=[ar2_out.opt()],
                )
            else:
                nc.gpsimd.dma_start(out=ar2_out, in_=ar2_in)
            g2 = stats.tile([C, 4], F32)
            nc.gpsimd.dma_start(out=g2, in_=ar2_out)

            a2 = stats.tile([C, 2], F32)
            c2 = stats.tile([C, 2], F32)
            gm2 = stats.tile([C, 1], F32)
            gvar2 = stats.tile([C, 1], F32)
            tmp2 = stats.tile([C, 1], F32)
            for hf in range(2):
                nc.scalar.mul(out=gm2, in_=g2[:, 2 * hf : 2 * hf + 1], mul=1.0 / N_CORES)
                nc.scalar.mul(
                    out=gvar2, in_=g2[:, 2 * hf + 1 : 2 * hf + 2], mul=1.0 / N_CORES
                )
                nc.vector.tensor_mul(out=tmp2, in0=gm2, in1=gm2)
                nc.vector.tensor_sub(out=gvar2, in0=gvar2, in1=tmp2)
                nc.scalar.activation(
                    out=gvar2, in_=gvar2,
                    func=mybir.ActivationFunctionType.Sqrt,
                    bias=eps_sb, scale=1.0,
                )
                nc.vector.reciprocal(out=tmp2, in_=gvar2)
                nc.vector.tensor_mul(
                    out=a2[:, hf : hf + 1], in0=tmp2, in1=bn2gb_sb[:, hf : hf + 1]
                )
                nc.vector.tensor_mul(out=tmp2, in0=gm2, in1=a2[:, hf : hf + 1])
                nc.vector.tensor_sub(
                    out=c2[:, hf : hf + 1],
                    in0=bn2gb_sb[:, 2 + hf : 3 + hf], in1=tmp2,
                )

            # ---- stage C: pw pass 2 + fused BN2+ReLU eviction + store -----
            for n in range(IMG_PER_CORE):
                for p0 in range(0, PIX_PER_IMG, PW_CHUNK):
                    sz = min(PW_CHUNK, PIX_PER_IMG - p0)
                    off = n * PIX_PER_IMG + p0
                    hsl = t_sb[:, off : off + sz]
                    for hf in range(2):
                        py = psum.tile([C, PW_CHUNK], F32, tag="ps")
                        nc.tensor.matmul(
                            py[:, :sz],
                            pwt_sb[:, hf * 128 : (hf + 1) * 128],
                            hsl.bitcast(F32R),
                            start=True, stop=True,
                        )
                        ot = yout.tile([C, PW_CHUNK], F32, tag="yo")
                        nc.scalar.activation(
                            out=ot[:, :sz], in_=py[:, :sz],
                            func=mybir.ActivationFunctionType.Relu,
                            bias=c2[:, hf : hf + 1],
                            scale=a2[:, hf : hf + 1],
                        )
                        nc.gpsimd.dma_start(
                            out=y_r[n, hf * 128 : (hf + 1) * 128, p0 : p0 + sz],
                            in_=ot[:, :sz],
                        )
    _legalize_waits(nc)
    return nc


def prepare(x, dw_w, dw_b, pw_w, pw_b, bn1_g, bn1_b, bn2_g, bn2_b, stride=1, **_):
    # dw_b / pw_b are absorbed by training-mode BN (they only shift the mean,
    # which BN subtracts) and are deliberately unused.
    x = np.asarray(x, dtype=np.float32)
    N = x.shape[0]
    assert x.shape == (16, C, H, W) and N == N_CORES * IMG_PER_CORE

    xp_full = np.zeros((N, C, HP, WP), dtype=np.float32)
    xp_full[:, :, 1 : 1 + H, 1 : 1 + W] = x

    dw9 = np.asarray(dw_w, dtype=np.float32).reshape(C, 9)
    dwdiag = np.zeros((C, 9, C), dtype=np.float32)
    idx = np.arange(C)
    for t in range(9):
        dwdiag[idx, t, idx] = dw9[:, t]

    pwt = np.asarray(pw_w, dtype=np.float32).reshape(O, C).T
    g1 = np.asarray(bn1_g, np.float32)
    b1 = np.asarray(bn1_b, np.float32)
    g2 = np.asarray(bn2_g, np.float32)
    b2 = np.asarray(bn2_b, np.float32)
    cst = np.concatenate(
        [
            dwdiag.reshape(C, 9 * C),
            pwt,
            g1[:, None], b1[:, None],
            g2[:128, None], g2[128:, None], b2[:128, None], b2[128:, None],
        ],
        axis=1,
    ).astype(np.float32)

    nc = _build_program()

    in_maps = []
    for k in range(N_CORES):
        in_maps.append(
            {
                "xp": np.ascontiguousarray(xp_full[IMG_PER_CORE * k : IMG_PER_CORE * (k + 1)]),
                "cst": cst,
            }
        )

    return nc, in_maps


def kernel(**inputs):
    nc, in_maps = prepare(**inputs)
    res = bass_utils.run_bass_kernel_spmd(
        nc, in_maps, core_ids=list(range(N_CORES))
    )
    out = np.concatenate([r["y"] for r in res.results], axis=0)
    return out


# revision 46
# speedup vs baseline: 1.2442x; 1.2442x over previous
"""Depthwise-separable conv block (dw3x3 + BN + ReLU + pw1x1 + BN + ReLU)
for Trainium2, data-parallel over batch across 8 NeuronCores with sync-BN
via two tiny AllReduces.

Key design points:
  - Depthwise conv = 9 PSUM-accumulated diagonal matmuls per pixel chunk
    (float32r => full PE rate at N>=256).
  - BN in training mode absorbs the conv biases (dw_b, pw_b shift the mean
    which BN subtracts), so they are dropped entirely.
  - BN1 folds to per-channel affine applied by one ScalarE op
    h = relu(a1*t + c1); BN2 folds into the PSUM eviction of the pointwise
    matmul: out = relu(a2*y + c2).
  - BN2 stats need a first pointwise pass (discarded into bn_stats); the
    final pass recomputes y after AllReduce-2 (PE is otherwise idle then).
"""

import numpy as np

import concourse.bass as bass
import concourse.tile as tile
import concourse.mybir as mybir
from concourse import bass_utils

N_CORES = 8
C = 128          # input channels (= SBUF partitions)
O = 256          # output channels
H = W = 112
HP = WP = 114    # zero-padded input
IMG_PER_CORE = 2
PIX_PER_IMG = H * W                 # 12544
PIX_TOTAL = IMG_PER_CORE * PIX_PER_IMG  # 25088
EPS = 1e-5

F32 = mybir.dt.float32
F32R = mybir.dt.float32r

DMA_ROWS = 16    # output rows per input DMA chunk (loads DMA_ROWS+2 rows)
SUB_ROWS = 4     # output rows per conv matmul chunk (N = 448)
PW_CHUNK = 512   # pixels per pointwise matmul (one PSUM bank of f32)

def _legalize_waits(nc):
    """Split multi-wait instructions: this walrus build's codegen accepts at
    most ONE sync wait per ISA instruction, while Tile's sem-assignment
    freely attaches several. Move all but one semaphore wait onto freshly
    inserted NoOps on the same engine directly before the instruction
    (waits are AND-semantics, so order is irrelevant)."""
    cnt = 0
    for bb in nc.main_func.blocks:
        new = []
        for ins in bb.instructions:
            si = ins.sync_info
            if si is not None and len(si.on_wait) > 1:
                sem_waits = [w for w in si.on_wait if w.sync_type == "semaphore"]
                other = [w for w in si.on_wait if w.sync_type != "semaphore"]
                keep = other + sem_waits[-1:] if not other else other
                move = sem_waits[:-1] if not other else sem_waits
                if len(keep) <= 1 and move:
                    for w in move:
                        cnt += 1
                        nop = mybir.InstNoOp(name=f"I-waitnop{cnt}", ins=[], outs=[])
                        nop.engine = ins.engine
                        nop.sync_info = mybir.SyncInfo(on_wait=[w], on_update=[])
                        new.append(nop)
                    ins.sync_info = mybir.SyncInfo(
                        on_wait=keep, on_update=list(si.on_update)
                    )
            new.append(ins)
        try:
            bb.instructions[:] = new
        except TypeError:
            bb.instructions = new
    return cnt


def _build_program(collectives=True):
    nc = bass.Bass(
        "TRN2",
        target_bir_lowering=False,
        debug=False,
        num_devices=N_CORES if collectives else 1,
    )

    # float32r = same 4-byte layout as f32 but lets the PE run matmuls at
    # full rate (fp32 proper is 4 cycles/row); the BIR verifier requires the
    # whole producer chain of a matmul operand to carry the f32r dtype.
    #
    # All constants are packed into ONE tensor so they arrive via one DMA on
    # one DMA lane: Matmult instructions only support a single sync wait, so
    # the first matmul cannot wait on separate weight+data DMA lanes.
    # Layout per channel row:
    #   [dwdiag 9*128 | pwT 256 | g1 b1 | g2h0 g2h1 b2h0 b2h1 | dw9 9]
    NCONST = 9 * C + O + 2 + 4 + 9
    xp = nc.dram_tensor("xp", (IMG_PER_CORE, C, HP, WP), F32R, kind="ExternalInput").ap()
    cst = nc.dram_tensor("cst", (C, NCONST), F32R, kind="ExternalInput").ap()
    y = nc.dram_tensor("y", (IMG_PER_CORE, O, H, W), F32, kind="ExternalOutput").ap()
    y_r = y.rearrange("n c h w -> n c (h w)")

    n_conv_chunks = IMG_PER_CORE * (H // SUB_ROWS)              # 56
    n_pw_chunks = IMG_PER_CORE * ((PIX_PER_IMG + PW_CHUNK - 1) // PW_CHUNK)  # 50

    with tile.TileContext(nc) as tc:
        with (
            tc.tile_pool(name="consts", bufs=1) as consts,
            tc.tile_pool(name="big", bufs=1) as big,
            tc.tile_pool(name="xin", bufs=3) as xin,
            tc.tile_pool(name="stats", bufs=1) as stats,
            tc.tile_pool(name="yout", bufs=4) as yout,
            # PSUM budget (8 banks): "ps" 4 bufs x 1 bank + "ps2" 2 bufs x 2
            tc.tile_pool(name="psum", bufs=4, space="PSUM") as psum,
            tc.tile_pool(name="dram", bufs=1, space="DRAM") as dram,
        ):
            # ---- constants (single DMA; see NCONST layout note above) -----
            cst_sb = consts.tile([C, NCONST], F32R)
            nc.gpsimd.dma_start(out=cst_sb, in_=cst)
            dwdiag_sb = cst_sb[:, 0 : 9 * C].rearrange("p (t c) -> p t c", t=9)
            pwt_sb = cst_sb[:, 9 * C : 9 * C + O]
            bn1gb_sb = cst_sb[:, 9 * C + O : 9 * C + O + 2].bitcast(F32)
            bn2gb_sb = cst_sb[:, 9 * C + O + 2 : 9 * C + O + 6].bitcast(F32)
            dw9_sb = cst_sb[:, 9 * C + O + 6 : 9 * C + O + 15].bitcast(F32)
            eps_sb = consts.tile([C, 1], F32)
            nc.vector.memset(eps_sb, EPS)

            # depthwise-conv output, SBUF-resident for the whole kernel
            t_sb = big.tile([C, PIX_TOTAL], F32)

            stats1 = stats.tile([C, n_conv_chunks, 6], F32)
            # Stage-B stats: one bn_stats per chunk over a two-bank PSUM tile
            # holding both output-channel halves -> [C, 2, 6] per chunk.
            stats2 = stats.tile([C, n_pw_chunks, 2, 6], F32)

            # ---- stage A: depthwise conv + BN1 partial stats --------------
            ci = 0
            for n in range(IMG_PER_CORE):
                for rblk in range(0, H, DMA_ROWS):
                    x_t = xin.tile([C, DMA_ROWS + 2, WP], F32R, tag="x")
                    # HWDGE (nc.sync): RTL descriptor generation, keeps the
                    # Pool engine free (SWDGE costs ~1us of Pool per DMA).
                    nc.sync.dma_start(
                        out=x_t, in_=xp[n, :, rblk : rblk + DMA_ROWS + 2, :]
                    )
                    for sr in range(0, DMA_ROWS, SUB_ROWS):
                        pt = psum.tile([C, SUB_ROWS * W], F32, tag="ps")
                        # Taps 0-7 on PE (diagonal matmuls into PSUM); tap 8
                        # as a scalar_tensor_tensor FMA on DVE after eviction
                        # (GPSIMD lacks TensorScalarPtr on this target).
                        for t9 in range(8):
                            di, dj = divmod(t9, 3)
                            rhs = x_t[:, sr + di : sr + di + SUB_ROWS, dj : dj + W]
                            nc.tensor.matmul(
                                pt,
                                dwdiag_sb[:, t9, :],
                                rhs,
                                start=(t9 == 0),
                                stop=(t9 == 7),
                            )
                        off = n * PIX_PER_IMG + (rblk + sr) * W
                        tsl = t_sb[:, off : off + SUB_ROWS * W]
                        tsl3 = tsl.rearrange("p (r w) -> p r w", r=SUB_ROWS)
                        nc.scalar.copy(out=tsl.bitcast(F32R), in_=pt)
                        t9 = 8
                        di, dj = divmod(t9, 3)
                        xs = x_t[
                            :, sr + di : sr + di + SUB_ROWS, dj : dj + W
                        ].bitcast(F32)
                        nc.vector.scalar_tensor_tensor(
                            out=tsl3.bitcast(F32R),
                            in0=xs,
                            scalar=dw9_sb[:, t9 : t9 + 1],
                            in1=tsl3,
                            op0=mybir.AluOpType.mult,
                            op1=mybir.AluOpType.add,
                        )
                        nc.vector.bn_stats(out=stats1[:, ci : ci + 1, :], in_=tsl)
                        ci += 1

            # ---- BN1: aggregate, AllReduce, fold --------------------------
            mv1 = stats.tile([C, 2], F32)
            nc.vector.bn_aggr(out=mv1, in_=stats1)
            ar1_sb = stats.tile([C, 2], F32)
            msq1 = stats.tile([C, 1], F32)
            nc.vector.tensor_copy(out=ar1_sb[:, 0:1], in_=mv1[:, 0:1])
            nc.vector.tensor_mul(out=msq1, in0=mv1[:, 0:1], in1=mv1[:, 0:1])
            nc.vector.tensor_add(out=ar1_sb[:, 1:2], in0=mv1[:, 1:2], in1=msq1)

            ar1_in = dram.tile([C, 2], F32)
            ar1_out = dram.tile([C, 2], F32)
            nc.gpsimd.dma_start(out=ar1_in, in_=ar1_sb)
            if collectives:
                nc.gpsimd.collective_compute(
                    "AllReduce",
                    mybir.AluOpType.add,
                    replica_groups=[list(range(N_CORES))],
                    ins=[ar1_in.opt()],
                    outs=[ar1_out.opt()],
                )
            else:  # timing-sim variant: skip the collective
                nc.gpsimd.dma_start(out=ar1_out, in_=ar1_in)
            g1 = stats.tile([C, 2], F32)
            nc.gpsimd.dma_start(out=g1, in_=ar1_out)

            gm1 = stats.tile([C, 1], F32)
            gex1 = stats.tile([C, 1], F32)
            gvar1 = stats.tile([C, 1], F32)
            a1 = stats.tile([C, 1], F32)
            c1 = stats.tile([C, 1], F32)
            nc.scalar.mul(out=gm1, in_=g1[:, 0:1], mul=1.0 / N_CORES)
            nc.scalar.mul(out=gex1, in_=g1[:, 1:2], mul=1.0 / N_CORES)
            nc.vector.tensor_mul(out=gvar1, in0=gm1, in1=gm1)
            nc.vector.tensor_sub(out=gvar1, in0=gex1, in1=gvar1)
            nc.scalar.activation(
                out=gvar1, in_=gvar1,
                func=mybir.ActivationFunctionType.Sqrt,
                bias=eps_sb, scale=1.0,
            )
            nc.vector.reciprocal(out=a1, in_=gvar1)
            nc.vector.tensor_mul(out=a1, in0=a1, in1=bn1gb_sb[:, 0:1])
            nc.vector.tensor_mul(out=c1, in0=gm1, in1=a1)
            nc.vector.tensor_sub(out=c1, in0=bn1gb_sb[:, 1:2], in1=c1)

            # ---- stage B: h = relu(a1*t + c1); pw pass 1 -> BN2 stats -----
            # All h-applies are issued first: they depend only on AR1, so ACT
            # streams through them without waiting on PE (interleaving them
            # with the PE-dependent stats ops would serialize ACT's FIFO on
            # per-chunk round-trips).
            chunks = []
            for n in range(IMG_PER_CORE):
                for p0 in range(0, PIX_PER_IMG, PW_CHUNK):
                    sz = min(PW_CHUNK, PIX_PER_IMG - p0)
                    off = n * PIX_PER_IMG + p0
                    chunks.append((off, sz))
                    hsl = t_sb[:, off : off + sz]
                    nc.scalar.activation(
                        out=hsl.bitcast(F32R), in_=hsl,
                        func=mybir.ActivationFunctionType.Relu,
                        bias=c1, scale=a1,
                    )

            for cj, (off, sz) in enumerate(chunks):
                hsl = t_sb[:, off : off + sz]
                py2 = psum.tile([C, 2, PW_CHUNK], F32, tag="ps2", bufs=2)
                for hf in range(2):
                    nc.tensor.matmul(
                        py2[:, hf, :sz],
                        pwt_sb[:, hf * 128 : (hf + 1) * 128],
                        hsl.bitcast(F32R),
                        start=True, stop=True,
                    )
                for hf in range(2):
                    nc.vector.bn_stats(
                        out=stats2[:, cj : cj + 1, hf, :], in_=py2[:, hf, :sz]
                    )

            # ---- BN2: aggregate, AllReduce, fold --------------------------
            mv2 = stats.tile([C, 2, 2], F32)
            ar2_sb = stats.tile([C, 4], F32)
            msq2 = stats.tile([C, 1], F32)
            for hf in range(2):
                nc.vector.bn_aggr(out=mv2[:, hf, :], in_=stats2[:, :, hf, :])
                nc.vector.tensor_copy(
                    out=ar2_sb[:, 2 * hf : 2 * hf + 1], in_=mv2[:, hf, 0:1]
                )
                nc.vector.tensor_mul(out=msq2, in0=mv2[:, hf, 0:1], in1=mv2[:, hf, 0:1])
                nc.vector.tensor_add(
                    out=ar2_sb[:, 2 * hf + 1 : 2 * hf + 2],
                    in0=mv2[:, hf, 1:2], in1=msq2,
                )

            ar2_in = dram.tile([C, 4], F32)
            ar2_out = dram.tile([C, 4], F32)
            nc.gpsimd.dma_start(out=ar2_in, in_=ar2_sb)
            if collectives:
                nc.gpsimd.collective_compute(
                    "AllReduce",
                    mybir.AluOpType.add,
                    replica_groups=[list(range(N_CORES))],
                    ins=[ar2_in.opt()],
                    outs=[ar2_out.opt()],
                )
            else:
                nc.gpsimd.dma_start(out=ar2_out, in_=ar2_in)
            g2 = stats.tile([C, 4], F32)
            nc.gpsimd.dma_start(out=g2, in_=ar2_out)

            a2 = stats.tile([C, 2], F32)
            c2 = stats.tile([C, 2], F32)
            gm2 = stats.tile([C, 1], F32)
            gvar2 = stats.tile([C, 1], F32)
            tmp2 = stats.tile([C, 1], F32)
            for hf in range(2):
                nc.scalar.mul(out=gm2, in_=g2[:, 2 * hf : 2 * hf + 1], mul=1.0 / N_CORES)
                nc.scalar.mul(
                    out=gvar2, in_=g2[:, 2 * hf + 1 : 2 * hf + 2], mul=1.0 / N_CORES
                )
                nc.vector.tensor_mul(out=tmp2, in0=gm2, in1=gm2)
                nc.vector.tensor_sub(out=gvar2, in0=gvar2, in1=tmp2)
                nc.scalar.activation(
                    out=gvar2, in_=gvar2,
                    func=mybir.ActivationFunctionType.Sqrt,
                    bias=eps_sb, scale=1.0,
                )
                nc.vector.reciprocal(out=tmp2, in_=gvar2)
                nc.vector.tensor_mul(
                    out=a2[:, hf : hf + 1], in0=tmp2, in1=bn2gb_sb[:, hf : hf + 1]
                )
                nc.vector.tensor_mul(out=tmp2, in0=gm2, in1=a2[:, hf : hf + 1])
                nc.vector.tensor_sub(
                    out=c2[:, hf : hf + 1],
                    in0=bn2gb_sb[:, 2 + hf : 3 + hf], in1=tmp2,
                )

            # ---- stage C: pw pass 2 + fused BN2+ReLU eviction + store -----
            # Two pixel-chunks per half are staged into one [C, 2*PW_CHUNK]
            # tile and shipped with a single 512KB DMA (fewer, bigger DMAs).
            for n in range(IMG_PER_CORE):
                for pblk in range(0, PIX_PER_IMG, 2 * PW_CHUNK):
                    blk_sz = min(2 * PW_CHUNK, PIX_PER_IMG - pblk)
                    for hf in range(2):
                        ot = yout.tile([C, 2 * PW_CHUNK], F32, tag="yo")
                        for sub in range(0, blk_sz, PW_CHUNK):
                            p0 = pblk + sub
                            sz = min(PW_CHUNK, PIX_PER_IMG - p0)
                            off = n * PIX_PER_IMG + p0
                            hsl = t_sb[:, off : off + sz]
                            py = psum.tile([C, PW_CHUNK], F32, tag="ps")
                            nc.tensor.matmul(
                                py[:, :sz],
                                pwt_sb[:, hf * 128 : (hf + 1) * 128],
                                hsl.bitcast(F32R),
                                start=True, stop=True,
                            )
                            nc.scalar.activation(
                                out=ot[:, sub : sub + sz], in_=py[:, :sz],
                                func=mybir.ActivationFunctionType.Relu,
                                bias=c2[:, hf : hf + 1],
                                scale=a2[:, hf : hf + 1],
                            )
                        nc.sync.dma_start(
                            out=y_r[n, hf * 128 : (hf + 1) * 128, pblk : pblk + blk_sz],
                            in_=ot[:, :blk_sz],
                        )
    _legalize_waits(nc)
    return nc


def prepare(x, dw_w, dw_b, pw_w, pw_b, bn1_g, bn1_b, bn2_g, bn2_b, stride=1, **_):
    # dw_b / pw_b are absorbed by training-mode BN (they only shift the mean,
    # which BN subtracts) and are deliberately unused.
    x = np.asarray(x, dtype=np.float32)
    N = x.shape[0]
    assert x.shape == (16, C, H, W) and N == N_CORES * IMG_PER_CORE

    xp_full = np.zeros((N, C, HP, WP), dtype=np.float32)
    xp_full[:, :, 1 : 1 + H, 1 : 1 + W] = x

    dw9 = np.asarray(dw_w, dtype=np.float32).reshape(C, 9)
    dwdiag = np.zeros((C, 9, C), dtype=np.float32)
    idx = np.arange(C)
    for t in range(9):
        dwdiag[idx, t, idx] = dw9[:, t]

    pwt = np.asarray(pw_w, dtype=np.float32).reshape(O, C).T
    g1 = np.asarray(bn1_g, np.float32)
    b1 = np.asarray(bn1_b, np.float32)
    g2 = np.asarray(bn2_g, np.float32)
    b2 = np.asarray(bn2_b, np.float32)
    cst = np.concatenate(
        [
            dwdiag.reshape(C, 9 * C),
            pwt,
            g1[:, None], b1[:, None],
            g2[:128, None], g2[128:, None], b2[:128, None], b2[128:, None],
            dw9,
        ],
        axis=1,
    ).astype(np.float32)

    nc = _build_program()

    in_maps = []
    for k in range(N_CORES):
        in_maps.append(
            {
                "xp": np.ascontiguousarray(xp_full[IMG_PER_CORE * k : IMG_PER_CORE * (k + 1)]),
                "cst": cst,
            }
        )

    return nc, in_maps


def kernel(**inputs):
    nc, in_maps = prepare(**inputs)
    res = bass_utils.run_bass_kernel_spmd(
        nc, in_maps, core_ids=list(range(N_CORES))
    )
    out = np.concatenate([r["y"] for r in res.results], axis=0)
    return out


# revision 50
# speedup vs baseline: 41.6908x; 33.5075x over previous
"""Depthwise-separable conv block (dw3x3 + BN + ReLU + pw1x1 + BN + ReLU)
for Trainium2, data-parallel over batch across 8 NeuronCores with sync-BN
via two tiny AllReduces.

Key design points:
  - Depthwise conv = 9 PSUM-accumulated diagonal matmuls per pixel chunk
    (float32r => full PE rate at N>=256).
  - BN in training mode absorbs the conv biases (dw_b, pw_b shift the mean
    which BN subtracts), so they are dropped entirely.
  - BN1 folds to per-channel affine applied by one ScalarE op
    h = relu(a1*t + c1); BN2 folds into the PSUM eviction of the pointwise
    matmul: out = relu(a2*y + c2).
  - BN2 stats need a first pointwise pass (discarded into bn_stats); the
    final pass recomputes y after AllReduce-2 (PE is otherwise idle then).
"""

import numpy as np

import concourse.bass as bass
import concourse.tile as tile
import concourse.mybir as mybir
from concourse import bass_utils

N_CORES = 8
C = 128          # input channels (= SBUF partitions)
O = 256          # output channels
H = W = 112
HP = WP = 114    # zero-padded input
IMG_PER_CORE = 2
PIX_PER_IMG = H * W                 # 12544
PIX_TOTAL = IMG_PER_CORE * PIX_PER_IMG  # 25088
EPS = 1e-5

F32 = mybir.dt.float32
F32R = mybir.dt.float32r

DMA_ROWS = 16    # output rows per input DMA chunk (loads DMA_ROWS+2 rows)
SUB_ROWS = 4     # output rows per conv matmul chunk (N = 448)
PW_CHUNK = 512   # pixels per pointwise matmul (one PSUM bank of f32)

def _legalize_waits(nc):
    """Split multi-wait instructions: this walrus build's codegen accepts at
    most ONE sync wait per ISA instruction, while Tile's sem-assignment
    freely attaches several. Move all but one semaphore wait onto freshly
    inserted NoOps on the same engine directly before the instruction
    (waits are AND-semantics, so order is irrelevant)."""
    cnt = 0
    for bb in nc.main_func.blocks:
        new = []
        for ins in bb.instructions:
            si = ins.sync_info
            if si is not None and len(si.on_wait) > 1:
                sem_waits = [w for w in si.on_wait if w.sync_type == "semaphore"]
                other = [w for w in si.on_wait if w.sync_type != "semaphore"]
                keep = other + sem_waits[-1:] if not other else other
                move = sem_waits[:-1] if not other else sem_waits
                if len(keep) <= 1 and move:
                    for w in move:
                        cnt += 1
                        nop = mybir.InstNoOp(name=f"I-waitnop{cnt}", ins=[], outs=[])
                        nop.engine = ins.engine
                        nop.sync_info = mybir.SyncInfo(on_wait=[w], on_update=[])
                        new.append(nop)
                    ins.sync_info = mybir.SyncInfo(
                        on_wait=keep, on_update=list(si.on_update)
                    )
            new.append(ins)
        try:
            bb.instructions[:] = new
        except TypeError:
            bb.instructions = new
    return cnt


def _build_program(collectives=True, repeat=1):
    nc = bass.Bass(
        "TRN2",
        target_bir_lowering=False,
        debug=False,
        num_devices=N_CORES if collectives else 1,
    )

    # float32r = same 4-byte layout as f32 but lets the PE run matmuls at
    # full rate (fp32 proper is 4 cycles/row); the BIR verifier requires the
    # whole producer chain of a matmul operand to carry the f32r dtype.
    #
    # All constants are packed into ONE tensor so they arrive via one DMA on
    # one DMA lane: Matmult instructions only support a single sync wait, so
    # the first matmul cannot wait on separate weight+data DMA lanes.
    # Layout per channel row:
    #   [dwdiag 9*128 | pwT 256 | g1 b1 | g2h0 g2h1 b2h0 b2h1 | dw9 9]
    NCONST = 9 * C + O + 2 + 4 + 9
    xp = nc.dram_tensor("xp", (IMG_PER_CORE, C, HP, WP), F32R, kind="ExternalInput").ap()
    cst = nc.dram_tensor("cst", (C, NCONST), F32R, kind="ExternalInput").ap()
    y = nc.dram_tensor("y", (IMG_PER_CORE, O, H, W), F32, kind="ExternalOutput").ap()
    y_r = y.rearrange("n c h w -> n c (h w)")

    n_conv_chunks = IMG_PER_CORE * (H // SUB_ROWS)              # 56
    n_pw_chunks = IMG_PER_CORE * ((PIX_PER_IMG + PW_CHUNK - 1) // PW_CHUNK)  # 50

    # `repeat` re-emits the whole body (benchmarking aid: the wall-time slope
    # between repeat=1 and repeat=K programs isolates the on-device time from
    # the ~80ms axon dispatch overhead).
    with tile.TileContext(nc) as tc:
      for _rep in range(repeat):
        with (
            tc.tile_pool(name="consts", bufs=1) as consts,
            tc.tile_pool(name="big", bufs=1) as big,
            tc.tile_pool(name="xin", bufs=3) as xin,
            tc.tile_pool(name="stats", bufs=1) as stats,
            tc.tile_pool(name="yout", bufs=4) as yout,
            # PSUM budget (8 banks): "ps" 4 bufs x 1 bank + "ps2" 2 bufs x 2
            tc.tile_pool(name="psum", bufs=4, space="PSUM") as psum,
            tc.tile_pool(name="dram", bufs=1, space="DRAM") as dram,
        ):
            # ---- constants (single DMA; see NCONST layout note above) -----
            cst_sb = consts.tile([C, NCONST], F32R)
            nc.gpsimd.dma_start(out=cst_sb, in_=cst)
            dwdiag_sb = cst_sb[:, 0 : 9 * C].rearrange("p (t c) -> p t c", t=9)
            pwt_sb = cst_sb[:, 9 * C : 9 * C + O]
            bn1gb_sb = cst_sb[:, 9 * C + O : 9 * C + O + 2].bitcast(F32)
            bn2gb_sb = cst_sb[:, 9 * C + O + 2 : 9 * C + O + 6].bitcast(F32)
            dw9_sb = cst_sb[:, 9 * C + O + 6 : 9 * C + O + 15].bitcast(F32)
            eps_sb = consts.tile([C, 1], F32)
            nc.vector.memset(eps_sb, EPS)

            # depthwise-conv output, SBUF-resident for the whole kernel
            t_sb = big.tile([C, PIX_TOTAL], F32)

            stats1 = stats.tile([C, n_conv_chunks, 6], F32)
            # Stage-B stats: one bn_stats per chunk over a two-bank PSUM tile
            # holding both output-channel halves -> [C, 2, 6] per chunk.
            stats2 = stats.tile([C, n_pw_chunks, 2, 6], F32)

            # ---- stage A: depthwise conv + BN1 partial stats --------------
            ci = 0
            for n in range(IMG_PER_CORE):
                for rblk in range(0, H, DMA_ROWS):
                    x_t = xin.tile([C, DMA_ROWS + 2, WP], F32R, tag="x")
                    # HWDGE (nc.sync): RTL descriptor generation, keeps the
                    # Pool engine free (SWDGE costs ~1us of Pool per DMA).
                    nc.sync.dma_start(
                        out=x_t, in_=xp[n, :, rblk : rblk + DMA_ROWS + 2, :]
                    )
                    for sr in range(0, DMA_ROWS, SUB_ROWS):
                        pt = psum.tile([C, SUB_ROWS * W], F32, tag="ps")
                        # Taps 0-7 on PE (diagonal matmuls into PSUM); tap 8
                        # as a scalar_tensor_tensor FMA on DVE after eviction
                        # (GPSIMD lacks TensorScalarPtr on this target).
                        for t9 in range(8):
                            di, dj = divmod(t9, 3)
                            rhs = x_t[:, sr + di : sr + di + SUB_ROWS, dj : dj + W]
                            nc.tensor.matmul(
                                pt,
                                dwdiag_sb[:, t9, :],
                                rhs,
                                start=(t9 == 0),
                                stop=(t9 == 7),
                            )
                        off = n * PIX_PER_IMG + (rblk + sr) * W
                        tsl = t_sb[:, off : off + SUB_ROWS * W]
                        tsl3 = tsl.rearrange("p (r w) -> p r w", r=SUB_ROWS)
                        nc.scalar.copy(out=tsl.bitcast(F32R), in_=pt)
                        t9 = 8
                        di, dj = divmod(t9, 3)
                        xs = x_t[
                            :, sr + di : sr + di + SUB_ROWS, dj : dj + W
                        ].bitcast(F32)
                        nc.vector.scalar_tensor_tensor(
                            out=tsl3.bitcast(F32R),
                            in0=xs,
                            scalar=dw9_sb[:, t9 : t9 + 1],
                            in1=tsl3,
                            op0=mybir.AluOpType.mult,
                            op1=mybir.AluOpType.add,
                        )
                        nc.vector.bn_stats(out=stats1[:, ci : ci + 1, :], in_=tsl)
                        ci += 1

            # ---- BN1: aggregate, AllReduce, fold --------------------------
            mv1 = stats.tile([C, 2], F32)
            nc.vector.bn_aggr(out=mv1, in_=stats1)
            ar1_sb = stats.tile([C, 2], F32)
            msq1 = stats.tile([C, 1], F32)
            nc.vector.tensor_copy(out=ar1_sb[:, 0:1], in_=mv1[:, 0:1])
            nc.vector.tensor_mul(out=msq1, in0=mv1[:, 0:1], in1=mv1[:, 0:1])
            nc.vector.tensor_add(out=ar1_sb[:, 1:2], in0=mv1[:, 1:2], in1=msq1)

            ar1_in = dram.tile([C, 2], F32)
            ar1_out = dram.tile([C, 2], F32)
            nc.gpsimd.dma_start(out=ar1_in, in_=ar1_sb)
            if collectives:
                nc.gpsimd.collective_compute(
                    "AllReduce",
                    mybir.AluOpType.add,
                    replica_groups=[list(range(N_CORES))],
                    ins=[ar1_in.opt()],
                    outs=[ar1_out.opt()],
                )
            else:  # timing-sim variant: skip the collective
                nc.gpsimd.dma_start(out=ar1_out, in_=ar1_in)
            g1 = stats.tile([C, 2], F32)
            nc.gpsimd.dma_start(out=g1, in_=ar1_out)

            gm1 = stats.tile([C, 1], F32)
            gex1 = stats.tile([C, 1], F32)
            gvar1 = stats.tile([C, 1], F32)
            a1 = stats.tile([C, 1], F32)
            c1 = stats.tile([C, 1], F32)
            nc.scalar.mul(out=gm1, in_=g1[:, 0:1], mul=1.0 / N_CORES)
            nc.scalar.mul(out=gex1, in_=g1[:, 1:2], mul=1.0 / N_CORES)
            nc.vector.tensor_mul(out=gvar1, in0=gm1, in1=gm1)
            nc.vector.tensor_sub(out=gvar1, in0=gex1, in1=gvar1)
            nc.scalar.activation(
                out=gvar1, in_=gvar1,
                func=mybir.ActivationFunctionType.Sqrt,
                bias=eps_sb, scale=1.0,
            )
            nc.vector.reciprocal(out=a1, in_=gvar1)
            nc.vector.tensor_mul(out=a1, in0=a1, in1=bn1gb_sb[:, 0:1])
            nc.vector.tensor_mul(out=c1, in0=gm1, in1=a1)
            nc.vector.tensor_sub(out=c1, in0=bn1gb_sb[:, 1:2], in1=c1)

            # ---- stage B: h = relu(a1*t + c1); pw pass 1 -> BN2 stats -----
            # All h-applies are issued first: they depend only on AR1, so ACT
            # streams through them without waiting on PE (interleaving them
            # with the PE-dependent stats ops would serialize ACT's FIFO on
            # per-chunk round-trips).
            chunks = []
            for n in range(IMG_PER_CORE):
                for p0 in range(0, PIX_PER_IMG, PW_CHUNK):
                    sz = min(PW_CHUNK, PIX_PER_IMG - p0)
                    off = n * PIX_PER_IMG + p0
                    chunks.append((off, sz))
                    hsl = t_sb[:, off : off + sz]
                    nc.scalar.activation(
                        out=hsl.bitcast(F32R), in_=hsl,
                        func=mybir.ActivationFunctionType.Relu,
                        bias=c1, scale=a1,
                    )

            for cj, (off, sz) in enumerate(chunks):
                hsl = t_sb[:, off : off + sz]
                py2 = psum.tile([C, 2, PW_CHUNK], F32, tag="ps2", bufs=2)
                for hf in range(2):
                    nc.tensor.matmul(
                        py2[:, hf, :sz],
                        pwt_sb[:, hf * 128 : (hf + 1) * 128],
                        hsl.bitcast(F32R),
                        start=True, stop=True,
                    )
                for hf in range(2):
                    nc.vector.bn_stats(
                        out=stats2[:, cj : cj + 1, hf, :], in_=py2[:, hf, :sz]
                    )

            # ---- BN2: aggregate, AllReduce, fold --------------------------
            mv2 = stats.tile([C, 2, 2], F32)
            ar2_sb = stats.tile([C, 4], F32)
            msq2 = stats.tile([C, 1], F32)
            for hf in range(2):
                nc.vector.bn_aggr(out=mv2[:, hf, :], in_=stats2[:, :, hf, :])
                nc.vector.tensor_copy(
                    out=ar2_sb[:, 2 * hf : 2 * hf + 1], in_=mv2[:, hf, 0:1]
                )
                nc.vector.tensor_mul(out=msq2, in0=mv2[:, hf, 0:1], in1=mv2[:, hf, 0:1])
                nc.vector.tensor_add(
                    out=ar2_sb[:, 2 * hf + 1 : 2 * hf + 2],
                    in0=mv2[:, hf, 1:2], in1=msq2,
                )

            ar2_in = dram.tile([C, 4], F32)
            ar2_out = dram.tile([C, 4], F32)
            nc.gpsimd.dma_start(out=ar2_in, in_=ar2_sb)
            if collectives:
                nc.gpsimd.collective_compute(
                    "AllReduce",
                    mybir.AluOpType.add,
                    replica_groups=[list(range(N_CORES))],
                    ins=[ar2_in.opt()],
                    outs=[ar2_out.opt()],
                )
            else:
                nc.gpsimd.dma_start(out=ar2_out, in_=ar2_in)
            g2 = stats.tile([C, 4], F32)
            nc.gpsimd.dma_start(out=g2, in_=ar2_out)

            a2 = stats.tile([C, 2], F32)
            c2 = stats.tile([C, 2], F32)
            gm2 = stats.tile([C, 1], F32)
            gvar2 = stats.tile([C, 1], F32)
            tmp2 = stats.tile([C, 1], F32)
            for hf in range(2):
                nc.scalar.mul(out=gm2, in_=g2[:, 2 * hf : 2 * hf + 1], mul=1.0 / N_CORES)
                nc.scalar.mul(
                    out=gvar2, in_=g2[:, 2 * hf + 1 : 2 * hf + 2], mul=1.0 / N_CORES
                )
                nc.vector.tensor_mul(out=tmp2, in0=gm2, in1=gm2)
                nc.vector.tensor_sub(out=gvar2, in0=gvar2, in1=tmp2)
                nc.scalar.activation(
                    out=gvar2, in_=gvar2,
                    func=mybir.ActivationFunctionType.Sqrt,
                    bias=eps_sb, scale=1.0,
                )
                nc.vector.reciprocal(out=tmp2, in_=gvar2)
                nc.vector.tensor_mul(
                    out=a2[:, hf : hf + 1], in0=tmp2, in1=bn2gb_sb[:, hf : hf + 1]
                )
                nc.vector.tensor_mul(out=tmp2, in0=gm2, in1=a2[:, hf : hf + 1])
                nc.vector.tensor_sub(
                    out=c2[:, hf : hf + 1],
                    in0=bn2gb_sb[:, 2 + hf : 3 + hf], in1=tmp2,
                )

            # ---- stage C: pw pass 2 + fused BN2+ReLU eviction + store -----
            # Two pixel-chunks per half are staged into one [C, 2*PW_CHUNK]
            # tile and shipped with a single 512KB DMA (fewer, bigger DMAs).
            for n in range(IMG_PER_CORE):
                for pblk in range(0, PIX_PER_IMG, 2 * PW_CHUNK):
                    blk_sz = min(2 * PW_CHUNK, PIX_PER_IMG - pblk)
                    for hf in range(2):
                        ot = yout.tile([C, 2 * PW_CHUNK], F32, tag="yo")
                        for sub in range(0, blk_sz, PW_CHUNK):
                            p0 = pblk + sub
                            sz = min(PW_CHUNK, PIX_PER_IMG - p0)
                            off = n * PIX_PER_IMG + p0
                            hsl = t_sb[:, off : off + sz]
                            py = psum.tile([C, PW_CHUNK], F32, tag="ps")
                            nc.tensor.matmul(
                                py[:, :sz],
                                pwt_sb[:, hf * 128 : (hf + 1) * 128],
                                hsl.bitcast(F32R),
                                start=True, stop=True,
                            )
                            nc.scalar.activation(
                                out=ot[:, sub : sub + sz], in_=py[:, :sz],
                                func=mybir.ActivationFunctionType.Relu,
                                bias=c2[:, hf : hf + 1],
                                scale=a2[:, hf : hf + 1],
                            )
                        nc.sync.dma_start(
                            out=y_r[n, hf * 128 : (hf + 1) * 128, pblk : pblk + blk_sz],
                            in_=ot[:, :blk_sz],
                        )
    _legalize_waits(nc)
    return nc


_NC_CACHE = []


def prepare(x, dw_w, dw_b, pw_w, pw_b, bn1_g, bn1_b, bn2_g, bn2_b, stride=1, **_):
    # dw_b / pw_b are absorbed by training-mode BN (they only shift the mean,
    # which BN subtracts) and are deliberately unused.
    x = np.asarray(x, dtype=np.float32)
    N = x.shape[0]
    assert x.shape == (16, C, H, W) and N == N_CORES * IMG_PER_CORE

    xp_full = np.zeros((N, C, HP, WP), dtype=np.float32)
    xp_full[:, :, 1 : 1 + H, 1 : 1 + W] = x

    dw9 = np.asarray(dw_w, dtype=np.float32).reshape(C, 9)
    dwdiag = np.zeros((C, 9, C), dtype=np.float32)
    idx = np.arange(C)
    for t in range(9):
        dwdiag[idx, t, idx] = dw9[:, t]

    pwt = np.asarray(pw_w, dtype=np.float32).reshape(O, C).T
    g1 = np.asarray(bn1_g, np.float32)
    b1 = np.asarray(bn1_b, np.float32)
    g2 = np.asarray(bn2_g, np.float32)
    b2 = np.asarray(bn2_b, np.float32)
    cst = np.concatenate(
        [
            dwdiag.reshape(C, 9 * C),
            pwt,
            g1[:, None], b1[:, None],
            g2[:128, None], g2[128:, None], b2[:128, None], b2[128:, None],
            dw9,
        ],
        axis=1,
    ).astype(np.float32)

    if not _NC_CACHE:
        _NC_CACHE.append(_build_program())
    nc = _NC_CACHE[0]

    in_maps = []
    for k in range(N_CORES):
        in_maps.append(
            {
                "xp": np.ascontiguousarray(xp_full[IMG_PER_CORE * k : IMG_PER_CORE * (k + 1)]),
                "cst": cst,
            }
        )

    return nc, in_maps


def kernel(**inputs):
    nc, in_maps = prepare(**inputs)
    res = bass_utils.run_bass_kernel_spmd(
        nc, in_maps, core_ids=list(range(N_CORES))
    )
    out = np.concatenate([r["y"] for r in res.results], axis=0)
    return out


# revision 56
# speedup vs baseline: 306.2241x; 7.3451x over previous
"""Depthwise-separable conv block (dw3x3 + BN + ReLU + pw1x1 + BN + ReLU)
for Trainium2, data-parallel over batch across 8 NeuronCores with sync-BN
via two tiny AllReduces.

Key design points:
  - Depthwise conv = 9 PSUM-accumulated diagonal matmuls per pixel chunk
    (float32r => full PE rate at N>=256).
  - BN in training mode absorbs the conv biases (dw_b, pw_b shift the mean
    which BN subtracts), so they are dropped entirely.
  - BN1 folds to per-channel affine applied by one ScalarE op
    h = relu(a1*t + c1); BN2 folds into the PSUM eviction of the pointwise
    matmul: out = relu(a2*y + c2).
  - BN2 stats need a first pointwise pass (discarded into bn_stats); the
    final pass recomputes y after AllReduce-2 (PE is otherwise idle then).
"""

import numpy as np

import concourse.bass as bass
import concourse.tile as tile
import concourse.mybir as mybir
from concourse import bass_utils

N_CORES = 8
C = 128          # input channels (= SBUF partitions)
O = 256          # output channels
H = W = 112
HP = WP = 114    # zero-padded input
IMG_PER_CORE = 2
PIX_PER_IMG = H * W                 # 12544
PIX_TOTAL = IMG_PER_CORE * PIX_PER_IMG  # 25088
EPS = 1e-5

F32 = mybir.dt.float32
F32R = mybir.dt.float32r

DMA_ROWS = 16    # output rows per input DMA chunk (loads DMA_ROWS+2 rows)
SUB_ROWS = 4     # output rows per conv matmul chunk (N = 448)
PW_CHUNK = 512   # pixels per pointwise matmul (one PSUM bank of f32)

def _legalize_waits(nc):
    """Split multi-wait instructions: this walrus build's codegen accepts at
    most ONE sync wait per ISA instruction, while Tile's sem-assignment
    freely attaches several. Move all but one semaphore wait onto freshly
    inserted NoOps on the same engine directly before the instruction
    (waits are AND-semantics, so order is irrelevant)."""
    cnt = 0
    for bb in nc.main_func.blocks:
        new = []
        for ins in bb.instructions:
            si = ins.sync_info
            if si is not None and len(si.on_wait) > 1:
                sem_waits = [w for w in si.on_wait if w.sync_type == "semaphore"]
                other = [w for w in si.on_wait if w.sync_type != "semaphore"]
                keep = other + sem_waits[-1:] if not other else other
                move = sem_waits[:-1] if not other else sem_waits
                if len(keep) <= 1 and move:
                    for w in move:
                        cnt += 1
                        nop = mybir.InstNoOp(name=f"I-waitnop{cnt}", ins=[], outs=[])
                        nop.engine = ins.engine
                        nop.sync_info = mybir.SyncInfo(on_wait=[w], on_update=[])
                        new.append(nop)
                    ins.sync_info = mybir.SyncInfo(
                        on_wait=keep, on_update=list(si.on_update)
                    )
            new.append(ins)
        try:
            bb.instructions[:] = new
        except TypeError:
            bb.instructions = new
    return cnt


def _build_program(collectives=True, repeat=1):
    nc = bass.Bass(
        "TRN2",
        target_bir_lowering=False,
        debug=False,
        num_devices=N_CORES if collectives else 1,
    )

    # float32r = same 4-byte layout as f32 but lets the PE run matmuls at
    # full rate (fp32 proper is 4 cycles/row); the BIR verifier requires the
    # whole producer chain of a matmul operand to carry the f32r dtype.
    #
    # All constants are packed into ONE tensor so they arrive via one DMA on
    # one DMA lane: Matmult instructions only support a single sync wait, so
    # the first matmul cannot wait on separate weight+data DMA lanes.
    # Layout per channel row:
    #   [dwdiag 9*128 | pwT 256 | g1 b1 | g2h0 g2h1 b2h0 b2h1 | dw9 9]
    NCONST = 9 * C + O + 2 + 4 + 9
    xp = nc.dram_tensor("xp", (IMG_PER_CORE, C, HP, WP), F32R, kind="ExternalInput").ap()
    cst = nc.dram_tensor("cst", (C, NCONST), F32R, kind="ExternalInput").ap()
    y = nc.dram_tensor("y", (IMG_PER_CORE, O, H, W), F32, kind="ExternalOutput").ap()
    y_r = y.rearrange("n c h w -> n c (h w)")

    n_conv_chunks = IMG_PER_CORE * (H // SUB_ROWS)              # 56
    n_pw_chunks = IMG_PER_CORE * ((PIX_PER_IMG + PW_CHUNK - 1) // PW_CHUNK)  # 50

    # `repeat` re-emits the whole body (benchmarking aid: the wall-time slope
    # between repeat=1 and repeat=K programs isolates the on-device time from
    # the ~80ms axon dispatch overhead).
    with tile.TileContext(nc) as tc:
      for _rep in range(repeat):
        with (
            tc.tile_pool(name="consts", bufs=1) as consts,
            tc.tile_pool(name="big", bufs=1) as big,
            tc.tile_pool(name="xin", bufs=3) as xin,
            tc.tile_pool(name="stats", bufs=1) as stats,
            tc.tile_pool(name="yout", bufs=6) as yout,
            # PSUM budget (8 banks): "ps" 4 bufs x 1 bank + "ps2" 2 bufs x 2
            tc.tile_pool(name="psum", bufs=4, space="PSUM") as psum,
            tc.tile_pool(name="dram", bufs=1, space="DRAM") as dram,
        ):
            # ---- constants (single DMA; see NCONST layout note above) -----
            cst_sb = consts.tile([C, NCONST], F32R)
            nc.gpsimd.dma_start(out=cst_sb, in_=cst)
            dwdiag_sb = cst_sb[:, 0 : 9 * C].rearrange("p (t c) -> p t c", t=9)
            pwt_sb = cst_sb[:, 9 * C : 9 * C + O]
            bn1gb_sb = cst_sb[:, 9 * C + O : 9 * C + O + 2].bitcast(F32)
            bn2gb_sb = cst_sb[:, 9 * C + O + 2 : 9 * C + O + 6].bitcast(F32)
            dw9_sb = cst_sb[:, 9 * C + O + 6 : 9 * C + O + 15].bitcast(F32)
            eps_sb = consts.tile([C, 1], F32)
            nc.vector.memset(eps_sb, EPS)

            # depthwise-conv output, SBUF-resident for the whole kernel
            t_sb = big.tile([C, PIX_TOTAL], F32)

            stats1 = stats.tile([C, n_conv_chunks, 6], F32)
            # Stage-B stats: one bn_stats per chunk-half on DVE.
            stats2 = stats.tile([C, n_pw_chunks, 2, 6], F32)

            # ---- stage A: depthwise conv + BN1 partial stats --------------
            ci = 0
            for n in range(IMG_PER_CORE):
                for rblk in range(0, H, DMA_ROWS):
                    x_t = xin.tile([C, DMA_ROWS + 2, WP], F32R, tag="x")
                    # HWDGE (nc.sync): RTL descriptor generation, keeps the
                    # Pool engine free (SWDGE costs ~1us of Pool per DMA).
                    nc.sync.dma_start(
                        out=x_t, in_=xp[n, :, rblk : rblk + DMA_ROWS + 2, :]
                    )
                    for sr in range(0, DMA_ROWS, SUB_ROWS):
                        pt = psum.tile([C, SUB_ROWS * W], F32, tag="ps")
                        # Taps 0..6 on PE (diagonal matmuls into PSUM); the
                        # remaining taps are scalar_tensor_tensor FMAs on DVE
                        # after the eviction. Tap 7 alternates PE/DVE per
                        # chunk to balance the two engines (GPSIMD lacks
                        # TensorScalarPtr on this target).
                        n_pe_taps = 8 if ci % 2 == 0 else 7
                        for t9 in range(n_pe_taps):
                            di, dj = divmod(t9, 3)
                            rhs = x_t[:, sr + di : sr + di + SUB_ROWS, dj : dj + W]
                            nc.tensor.matmul(
                                pt,
                                dwdiag_sb[:, t9, :],
                                rhs,
                                start=(t9 == 0),
                                stop=(t9 == n_pe_taps - 1),
                            )
                        off = n * PIX_PER_IMG + (rblk + sr) * W
                        tsl = t_sb[:, off : off + SUB_ROWS * W]
                        tsl3 = tsl.rearrange("p (r w) -> p r w", r=SUB_ROWS)
                        nc.scalar.copy(out=tsl.bitcast(F32R), in_=pt)
                        for t9 in range(n_pe_taps, 9):
                            di, dj = divmod(t9, 3)
                            xs = x_t[
                                :, sr + di : sr + di + SUB_ROWS, dj : dj + W
                            ].bitcast(F32)
                            nc.vector.scalar_tensor_tensor(
                                out=tsl3.bitcast(F32R),
                                in0=xs,
                                scalar=dw9_sb[:, t9 : t9 + 1],
                                in1=tsl3,
                                op0=mybir.AluOpType.mult,
                                op1=mybir.AluOpType.add,
                            )
                        nc.vector.bn_stats(out=stats1[:, ci : ci + 1, :], in_=tsl)
                        ci += 1

            # ---- BN1: aggregate, AllReduce, fold --------------------------
            mv1 = stats.tile([C, 2], F32)
            nc.vector.bn_aggr(out=mv1, in_=stats1)
            ar1_sb = stats.tile([C, 2], F32)
            msq1 = stats.tile([C, 1], F32)
            nc.vector.tensor_copy(out=ar1_sb[:, 0:1], in_=mv1[:, 0:1])
            nc.vector.tensor_mul(out=msq1, in0=mv1[:, 0:1], in1=mv1[:, 0:1])
            nc.vector.tensor_add(out=ar1_sb[:, 1:2], in0=mv1[:, 1:2], in1=msq1)

            ar1_in = dram.tile([C, 2], F32)
            ar1_out = dram.tile([C, 2], F32)
            nc.gpsimd.dma_start(out=ar1_in, in_=ar1_sb)
            if collectives:
                nc.gpsimd.collective_compute(
                    "AllReduce",
                    mybir.AluOpType.add,
                    replica_groups=[list(range(N_CORES))],
                    ins=[ar1_in.opt()],
                    outs=[ar1_out.opt()],
                )
            else:  # timing-sim variant: skip the collective
                nc.gpsimd.dma_start(out=ar1_out, in_=ar1_in)
            g1 = stats.tile([C, 2], F32)
            nc.gpsimd.dma_start(out=g1, in_=ar1_out)

            gm1 = stats.tile([C, 1], F32)
            gex1 = stats.tile([C, 1], F32)
            gvar1 = stats.tile([C, 1], F32)
            a1 = stats.tile([C, 1], F32)
            c1 = stats.tile([C, 1], F32)
            nc.scalar.mul(out=gm1, in_=g1[:, 0:1], mul=1.0 / N_CORES)
            nc.scalar.mul(out=gex1, in_=g1[:, 1:2], mul=1.0 / N_CORES)
            nc.vector.tensor_mul(out=gvar1, in0=gm1, in1=gm1)
            nc.vector.tensor_sub(out=gvar1, in0=gex1, in1=gvar1)
            nc.scalar.activation(
                out=gvar1, in_=gvar1,
                func=mybir.ActivationFunctionType.Sqrt,
                bias=eps_sb, scale=1.0,
            )
            nc.vector.reciprocal(out=a1, in_=gvar1)
            nc.vector.tensor_mul(out=a1, in0=a1, in1=bn1gb_sb[:, 0:1])
            nc.vector.tensor_mul(out=c1, in0=gm1, in1=a1)
            nc.vector.tensor_sub(out=c1, in0=bn1gb_sb[:, 1:2], in1=c1)

            # ---- stage B: h = relu(a1*t + c1); pw pass 1 -> BN2 stats -----
            # All h-applies are issued first: they depend only on AR1, so ACT
            # streams through them without waiting on PE (interleaving them
            # with the PE-dependent stats ops would serialize ACT's FIFO on
            # per-chunk round-trips).
            chunks = []
            for n in range(IMG_PER_CORE):
                for p0 in range(0, PIX_PER_IMG, PW_CHUNK):
                    sz = min(PW_CHUNK, PIX_PER_IMG - p0)
                    off = n * PIX_PER_IMG + p0
                    chunks.append((off, sz))
                    hsl = t_sb[:, off : off + sz]
                    nc.scalar.activation(
                        out=hsl.bitcast(F32R), in_=hsl,
                        func=mybir.ActivationFunctionType.Relu,
                        bias=c1, scale=a1,
                    )

            for cj, (off, sz) in enumerate(chunks):
                hsl = t_sb[:, off : off + sz]
                py2 = psum.tile([C, 2, PW_CHUNK], F32, tag="ps2", bufs=2)
                for hf in range(2):
                    nc.tensor.matmul(
                        py2[:, hf, :sz],
                        pwt_sb[:, hf * 128 : (hf + 1) * 128],
                        hsl.bitcast(F32R),
                        start=True, stop=True,
                    )
                for hf in range(2):
                    nc.vector.bn_stats(
                        out=stats2[:, cj : cj + 1, hf, :], in_=py2[:, hf, :sz]
                    )

            # ---- BN2: aggregate, AllReduce, fold --------------------------
            mv2 = stats.tile([C, 2, 2], F32)
            ar2_sb = stats.tile([C, 4], F32)
            msq2 = stats.tile([C, 1], F32)
            for hf in range(2):
                nc.vector.bn_aggr(out=mv2[:, hf, :], in_=stats2[:, :, hf, :])
                nc.vector.tensor_copy(
                    out=ar2_sb[:, 2 * hf : 2 * hf + 1], in_=mv2[:, hf, 0:1]
                )
                nc.vector.tensor_mul(out=msq2, in0=mv2[:, hf, 0:1], in1=mv2[:, hf, 0:1])
                nc.vector.tensor_add(
                    out=ar2_sb[:, 2 * hf + 1 : 2 * hf + 2],
                    in0=mv2[:, hf, 1:2], in1=msq2,
                )

            ar2_in = dram.tile([C, 4], F32)
            ar2_out = dram.tile([C, 4], F32)
            nc.gpsimd.dma_start(out=ar2_in, in_=ar2_sb)
            if collectives:
                nc.gpsimd.collective_compute(
                    "AllReduce",
                    mybir.AluOpType.add,
                    replica_groups=[list(range(N_CORES))],
                    ins=[ar2_in.opt()],
                    outs=[ar2_out.opt()],
                )
            else:
                nc.gpsimd.dma_start(out=ar2_out, in_=ar2_in)
            g2 = stats.tile([C, 4], F32)
            nc.gpsimd.dma_start(out=g2, in_=ar2_out)

            a2 = stats.tile([C, 2], F32)
            c2 = stats.tile([C, 2], F32)
            gm2 = stats.tile([C, 1], F32)
            gvar2 = stats.tile([C, 1], F32)
            tmp2 = stats.tile([C, 1], F32)
            for hf in range(2):
                nc.scalar.mul(out=gm2, in_=g2[:, 2 * hf : 2 * hf + 1], mul=1.0 / N_CORES)
                nc.scalar.mul(
                    out=gvar2, in_=g2[:, 2 * hf + 1 : 2 * hf + 2], mul=1.0 / N_CORES
                )
                nc.vector.tensor_mul(out=tmp2, in0=gm2, in1=gm2)
                nc.vector.tensor_sub(out=gvar2, in0=gvar2, in1=tmp2)
                nc.scalar.activation(
                    out=gvar2, in_=gvar2,
                    func=mybir.ActivationFunctionType.Sqrt,
                    bias=eps_sb, scale=1.0,
                )
                nc.vector.reciprocal(out=tmp2, in_=gvar2)
                nc.vector.tensor_mul(
                    out=a2[:, hf : hf + 1], in0=tmp2, in1=bn2gb_sb[:, hf : hf + 1]
                )
                nc.vector.tensor_mul(out=tmp2, in0=gm2, in1=a2[:, hf : hf + 1])
                nc.vector.tensor_sub(
                    out=c2[:, hf : hf + 1],
                    in0=bn2gb_sb[:, 2 + hf : 3 + hf], in1=tmp2,
                )

            # ---- stage C: pw pass 2 + fused BN2+ReLU eviction + store -----
            # Two pixel-chunks per half are staged into one [C, 2*PW_CHUNK]
            # tile and shipped with a single 512KB DMA (fewer, bigger DMAs).
            for n in range(IMG_PER_CORE):
                for pblk in range(0, PIX_PER_IMG, 2 * PW_CHUNK):
                    blk_sz = min(2 * PW_CHUNK, PIX_PER_IMG - pblk)
                    for hf in range(2):
                        ot = yout.tile([C, 2 * PW_CHUNK], F32, tag="yo")
                        for sub in range(0, blk_sz, PW_CHUNK):
                            p0 = pblk + sub
                            sz = min(PW_CHUNK, PIX_PER_IMG - p0)
                            off = n * PIX_PER_IMG + p0
                            hsl = t_sb[:, off : off + sz]
                            py = psum.tile([C, PW_CHUNK], F32, tag="ps")
                            nc.tensor.matmul(
                                py[:, :sz],
                                pwt_sb[:, hf * 128 : (hf + 1) * 128],
                                hsl.bitcast(F32R),
                                start=True, stop=True,
                            )
                            nc.scalar.activation(
                                out=ot[:, sub : sub + sz], in_=py[:, :sz],
                                func=mybir.ActivationFunctionType.Relu,
                                bias=c2[:, hf : hf + 1],
                                scale=a2[:, hf : hf + 1],
                            )
                        nc.sync.dma_start(
                            out=y_r[n, hf * 128 : (hf + 1) * 128, pblk : pblk + blk_sz],
                            in_=ot[:, :blk_sz],
                        )
    _legalize_waits(nc)
    return nc


_NC_CACHE = []


def prepare(x, dw_w, dw_b, pw_w, pw_b, bn1_g, bn1_b, bn2_g, bn2_b, stride=1, **_):
    # dw_b / pw_b are absorbed by training-mode BN (they only shift the mean,
    # which BN subtracts) and are deliberately unused.
    x = np.asarray(x, dtype=np.float32)
    N = x.shape[0]
    assert x.shape == (16, C, H, W) and N == N_CORES * IMG_PER_CORE

    xp_full = np.zeros((N, C, HP, WP), dtype=np.float32)
    xp_full[:, :, 1 : 1 + H, 1 : 1 + W] = x

    dw9 = np.asarray(dw_w, dtype=np.float32).reshape(C, 9)
    dwdiag = np.zeros((C, 9, C), dtype=np.float32)
    idx = np.arange(C)
    for t in range(9):
        dwdiag[idx, t, idx] = dw9[:, t]

    pwt = np.asarray(pw_w, dtype=np.float32).reshape(O, C).T
    g1 = np.asarray(bn1_g, np.float32)
    b1 = np.asarray(bn1_b, np.float32)
    g2 = np.asarray(bn2_g, np.float32)
    b2 = np.asarray(bn2_b, np.float32)
    cst = np.concatenate(
        [
            dwdiag.reshape(C, 9 * C),
            pwt,
            g1[:, None], b1[:, None],
            g2[:128, None], g2[128:, None], b2[:128, None], b2[128:, None],
            dw9,
        ],
        axis=1,
    ).astype(np.float32)

    if not _NC_CACHE:
        _NC_CACHE.append(_build_program())
    nc = _NC_CACHE[0]

    in_maps = []
    for k in range(N_CORES):
        in_maps.append(
            {
                "xp": np.ascontiguousarray(xp_full[IMG_PER_CORE * k : IMG_PER_CORE * (k + 1)]),
                "cst": cst,
            }
        )

    return nc, in_maps


def kernel(**inputs):
    nc, in_maps = prepare(**inputs)
    res = bass_utils.run_bass_kernel_spmd(
        nc, in_maps, core_ids=list(range(N_CORES))
    )
    out = np.concatenate([r["y"] for r in res.results], axis=0)
    return out


# revision 62
# speedup vs baseline: 319.3378x; 1.0428x over previous
"""Depthwise-separable conv block (dw3x3 + BN + ReLU + pw1x1 + BN + ReLU)
for Trainium2, data-parallel over batch across 8 NeuronCores with sync-BN
via two tiny AllReduces.

Key design points:
  - Depthwise conv = 9 PSUM-accumulated diagonal matmuls per pixel chunk
    (float32r => full PE rate at N>=256).
  - BN in training mode absorbs the conv biases (dw_b, pw_b shift the mean
    which BN subtracts), so they are dropped entirely.
  - BN1 folds to per-channel affine applied by one ScalarE op
    h = relu(a1*t + c1); BN2 folds into the PSUM eviction of the pointwise
    matmul: out = relu(a2*y + c2).
  - BN2 stats need a first pointwise pass (discarded into bn_stats); the
    final pass recomputes y after AllReduce-2 (PE is otherwise idle then).
"""

import numpy as np

import concourse.bass as bass
import concourse.tile as tile
import concourse.mybir as mybir
from concourse import bass_utils

N_CORES = 8
C = 128          # input channels (= SBUF partitions)
O = 256          # output channels
H = W = 112
HP = WP = 114    # zero-padded input
IMG_PER_CORE = 2
PIX_PER_IMG = H * W                 # 12544
PIX_TOTAL = IMG_PER_CORE * PIX_PER_IMG  # 25088
EPS = 1e-5

F32 = mybir.dt.float32
F32R = mybir.dt.float32r

DMA_ROWS = 16    # output rows per input DMA chunk (loads DMA_ROWS+2 rows)
SUB_ROWS = 4     # output rows per conv matmul chunk (N = 448)
PW_CHUNK = 512   # pixels per pointwise matmul (one PSUM bank of f32)

def _legalize_waits(nc):
    """Split multi-wait instructions: this walrus build's codegen accepts at
    most ONE sync wait per ISA instruction, while Tile's sem-assignment
    freely attaches several. Move all but one semaphore wait onto freshly
    inserted NoOps on the same engine directly before the instruction
    (waits are AND-semantics, so order is irrelevant)."""
    cnt = 0
    for bb in nc.main_func.blocks:
        new = []
        for ins in bb.instructions:
            si = ins.sync_info
            if si is not None and len(si.on_wait) > 1:
                sem_waits = [w for w in si.on_wait if w.sync_type == "semaphore"]
                other = [w for w in si.on_wait if w.sync_type != "semaphore"]
                keep = other + sem_waits[-1:] if not other else other
                move = sem_waits[:-1] if not other else sem_waits
                if len(keep) <= 1 and move:
                    for w in move:
                        cnt += 1
                        nop = mybir.InstNoOp(name=f"I-waitnop{cnt}", ins=[], outs=[])
                        nop.engine = ins.engine
                        nop.sync_info = mybir.SyncInfo(on_wait=[w], on_update=[])
                        new.append(nop)
                    ins.sync_info = mybir.SyncInfo(
                        on_wait=keep, on_update=list(si.on_update)
                    )
            new.append(ins)
        try:
            bb.instructions[:] = new
        except TypeError:
            bb.instructions = new
    return cnt


def _build_program(collectives=True, repeat=1):
    nc = bass.Bass(
        "TRN2",
        target_bir_lowering=False,
        debug=False,
        num_devices=N_CORES if collectives else 1,
    )

    # float32r = same 4-byte layout as f32 but lets the PE run matmuls at
    # full rate (fp32 proper is 4 cycles/row); the BIR verifier requires the
    # whole producer chain of a matmul operand to carry the f32r dtype.
    #
    # All constants are packed into ONE tensor so they arrive via one DMA on
    # one DMA lane: Matmult instructions only support a single sync wait, so
    # the first matmul cannot wait on separate weight+data DMA lanes.
    # Layout per channel row:
    #   [dwdiag 9*128 | pwT 256 | g1 b1 | g2h0 g2h1 b2h0 b2h1 | dw9 9]
    NCONST = 9 * C + O + 2 + 4 + 9
    xp = nc.dram_tensor("xp", (IMG_PER_CORE, C, HP, WP), F32R, kind="ExternalInput").ap()
    cst = nc.dram_tensor("cst", (C, NCONST), F32R, kind="ExternalInput").ap()
    y = nc.dram_tensor("y", (IMG_PER_CORE, O, H, W), F32, kind="ExternalOutput").ap()
    y_r = y.rearrange("n c h w -> n c (h w)")

    n_conv_chunks = IMG_PER_CORE * (H // SUB_ROWS)              # 56
    n_pw_chunks = IMG_PER_CORE * ((PIX_PER_IMG + PW_CHUNK - 1) // PW_CHUNK)  # 50

    # `repeat` re-emits the whole body (benchmarking aid: the wall-time slope
    # between repeat=1 and repeat=K programs isolates the on-device time from
    # the ~80ms axon dispatch overhead).
    with tile.TileContext(nc) as tc:
      for _rep in range(repeat):
        with (
            tc.tile_pool(name="consts", bufs=1) as consts,
            tc.tile_pool(name="big", bufs=1) as big,
            tc.tile_pool(name="xin", bufs=4) as xin,
            tc.tile_pool(name="stats", bufs=1) as stats,
            tc.tile_pool(name="yout", bufs=6) as yout,
            # PSUM budget (8 banks): "ps" 4 bufs x 1 bank + "ps2" 2 bufs x 2
            tc.tile_pool(name="psum", bufs=4, space="PSUM") as psum,
            tc.tile_pool(name="dram", bufs=1, space="DRAM") as dram,
        ):
            # ---- constants (single DMA; see NCONST layout note above) -----
            cst_sb = consts.tile([C, NCONST], F32R)
            nc.gpsimd.dma_start(out=cst_sb, in_=cst)
            dwdiag_sb = cst_sb[:, 0 : 9 * C].rearrange("p (t c) -> p t c", t=9)
            pwt_sb = cst_sb[:, 9 * C : 9 * C + O]
            bn1gb_sb = cst_sb[:, 9 * C + O : 9 * C + O + 2].bitcast(F32)
            bn2gb_sb = cst_sb[:, 9 * C + O + 2 : 9 * C + O + 6].bitcast(F32)
            dw9_sb = cst_sb[:, 9 * C + O + 6 : 9 * C + O + 15].bitcast(F32)
            eps_sb = consts.tile([C, 1], F32)
            nc.vector.memset(eps_sb, EPS)

            # depthwise-conv output, SBUF-resident for the whole kernel
            t_sb = big.tile([C, PIX_TOTAL], F32)

            stats1 = stats.tile([C, n_conv_chunks, 6], F32)
            # Stage-B stats: one bn_stats per chunk-half on DVE.
            stats2 = stats.tile([C, n_pw_chunks, 2, 6], F32)

            # ---- stage A: depthwise conv + BN1 partial stats --------------
            ci = 0
            for n in range(IMG_PER_CORE):
                for rblk in range(0, H, DMA_ROWS):
                    x_t = xin.tile([C, DMA_ROWS + 2, WP], F32R, tag="x")
                    # HWDGE (nc.sync): RTL descriptor generation, keeps the
                    # Pool engine free (SWDGE costs ~1us of Pool per DMA).
                    # The very first block is split so the first conv matmuls
                    # start after ~390KB instead of a full 1.05MB transfer.
                    if n == 0 and rblk == 0:
                        nc.sync.dma_start(
                            out=x_t[:, 0 : SUB_ROWS + 2, :],
                            in_=xp[n, :, 0 : SUB_ROWS + 2, :],
                        )
                        nc.sync.dma_start(
                            out=x_t[:, SUB_ROWS + 2 :, :],
                            in_=xp[n, :, SUB_ROWS + 2 : DMA_ROWS + 2, :],
                        )
                    else:
                        nc.sync.dma_start(
                            out=x_t, in_=xp[n, :, rblk : rblk + DMA_ROWS + 2, :]
                        )
                    for sr in range(0, DMA_ROWS, SUB_ROWS):
                        pt = psum.tile([C, SUB_ROWS * W], F32, tag="ps")
                        # Taps 0..6 on PE (diagonal matmuls into PSUM); the
                        # remaining taps are scalar_tensor_tensor FMAs on DVE
                        # after the eviction. Tap 7 alternates PE/DVE per
                        # chunk to balance the two engines (GPSIMD lacks
                        # TensorScalarPtr on this target).
                        n_pe_taps = 8 if ci % 3 == 0 else 7
                        for t9 in range(n_pe_taps):
                            di, dj = divmod(t9, 3)
                            rhs = x_t[:, sr + di : sr + di + SUB_ROWS, dj : dj + W]
                            nc.tensor.matmul(
                                pt,
                                dwdiag_sb[:, t9, :],
                                rhs,
                                start=(t9 == 0),
                                stop=(t9 == n_pe_taps - 1),
                            )
                        off = n * PIX_PER_IMG + (rblk + sr) * W
                        tsl = t_sb[:, off : off + SUB_ROWS * W]
                        tsl3 = tsl.rearrange("p (r w) -> p r w", r=SUB_ROWS)
                        nc.scalar.copy(out=tsl.bitcast(F32R), in_=pt)
                        for t9 in range(n_pe_taps, 9):
                            di, dj = divmod(t9, 3)
                            xs = x_t[
                                :, sr + di : sr + di + SUB_ROWS, dj : dj + W
                            ].bitcast(F32)
                            nc.vector.scalar_tensor_tensor(
                                out=tsl3.bitcast(F32R),
                                in0=xs,
                                scalar=dw9_sb[:, t9 : t9 + 1],
                                in1=tsl3,
                                op0=mybir.AluOpType.mult,
                                op1=mybir.AluOpType.add,
                            )
                        nc.vector.bn_stats(out=stats1[:, ci : ci + 1, :], in_=tsl)
                        ci += 1

            # ---- BN1: aggregate, AllReduce, fold --------------------------
            mv1 = stats.tile([C, 2], F32)
            nc.vector.bn_aggr(out=mv1, in_=stats1)
            ar1_sb = stats.tile([C, 2], F32)
            msq1 = stats.tile([C, 1], F32)
            nc.vector.tensor_copy(out=ar1_sb[:, 0:1], in_=mv1[:, 0:1])
            nc.vector.tensor_mul(out=msq1, in0=mv1[:, 0:1], in1=mv1[:, 0:1])
            nc.vector.tensor_add(out=ar1_sb[:, 1:2], in0=mv1[:, 1:2], in1=msq1)

            ar1_in = dram.tile([C, 2], F32)
            ar1_out = dram.tile([C, 2], F32)
            nc.gpsimd.dma_start(out=ar1_in, in_=ar1_sb)
            if collectives:
                nc.gpsimd.collective_compute(
                    "AllReduce",
                    mybir.AluOpType.add,
                    replica_groups=[list(range(N_CORES))],
                    ins=[ar1_in.opt()],
                    outs=[ar1_out.opt()],
                )
            else:  # timing-sim variant: skip the collective
                nc.gpsimd.dma_start(out=ar1_out, in_=ar1_in)
            g1 = stats.tile([C, 2], F32)
            nc.gpsimd.dma_start(out=g1, in_=ar1_out)

            gm1 = stats.tile([C, 1], F32)
            gex1 = stats.tile([C, 1], F32)
            gvar1 = stats.tile([C, 1], F32)
            a1 = stats.tile([C, 1], F32)
            c1 = stats.tile([C, 1], F32)
            nc.scalar.mul(out=gm1, in_=g1[:, 0:1], mul=1.0 / N_CORES)
            nc.scalar.mul(out=gex1, in_=g1[:, 1:2], mul=1.0 / N_CORES)
            nc.vector.tensor_mul(out=gvar1, in0=gm1, in1=gm1)
            nc.vector.tensor_sub(out=gvar1, in0=gex1, in1=gvar1)
            nc.scalar.activation(
                out=gvar1, in_=gvar1,
                func=mybir.ActivationFunctionType.Sqrt,
                bias=eps_sb, scale=1.0,
            )
            nc.vector.reciprocal(out=a1, in_=gvar1)
            nc.vector.tensor_mul(out=a1, in0=a1, in1=bn1gb_sb[:, 0:1])
            nc.vector.tensor_mul(out=c1, in0=gm1, in1=a1)
            nc.vector.tensor_sub(out=c1, in0=bn1gb_sb[:, 1:2], in1=c1)

            # ---- stage B: h = relu(a1*t + c1); pw pass 1 -> BN2 stats -----
            # All h-applies are issued first: they depend only on AR1, so ACT
            # streams through them without waiting on PE (interleaving them
            # with the PE-dependent stats ops would serialize ACT's FIFO on
            # per-chunk round-trips).
            chunks = []
            for n in range(IMG_PER_CORE):
                for p0 in range(0, PIX_PER_IMG, PW_CHUNK):
                    sz = min(PW_CHUNK, PIX_PER_IMG - p0)
                    off = n * PIX_PER_IMG + p0
                    chunks.append((off, sz))
                    hsl = t_sb[:, off : off + sz]
                    nc.scalar.activation(
                        out=hsl.bitcast(F32R), in_=hsl,
                        func=mybir.ActivationFunctionType.Relu,
                        bias=c1, scale=a1,
                    )

            for cj, (off, sz) in enumerate(chunks):
                hsl = t_sb[:, off : off + sz]
                py2 = psum.tile([C, 2, PW_CHUNK], F32, tag="ps2", bufs=2)
                for hf in range(2):
                    nc.tensor.matmul(
                        py2[:, hf, :sz],
                        pwt_sb[:, hf * 128 : (hf + 1) * 128],
                        hsl.bitcast(F32R),
                        start=True, stop=True,
                    )
                for hf in range(2):
                    nc.vector.bn_stats(
                        out=stats2[:, cj : cj + 1, hf, :], in_=py2[:, hf, :sz]
                    )

            # ---- BN2: aggregate, AllReduce, fold --------------------------
            mv2 = stats.tile([C, 2, 2], F32)
            ar2_sb = stats.tile([C, 4], F32)
            msq2 = stats.tile([C, 1], F32)
            for hf in range(2):
                nc.vector.bn_aggr(out=mv2[:, hf, :], in_=stats2[:, :, hf, :])
                nc.vector.tensor_copy(
                    out=ar2_sb[:, 2 * hf : 2 * hf + 1], in_=mv2[:, hf, 0:1]
                )
                nc.vector.tensor_mul(out=msq2, in0=mv2[:, hf, 0:1], in1=mv2[:, hf, 0:1])
                nc.vector.tensor_add(
                    out=ar2_sb[:, 2 * hf + 1 : 2 * hf + 2],
                    in0=mv2[:, hf, 1:2], in1=msq2,
                )

            ar2_in = dram.tile([C, 4], F32)
            ar2_out = dram.tile([C, 4], F32)
            nc.gpsimd.dma_start(out=ar2_in, in_=ar2_sb)
            if collectives:
                nc.gpsimd.collective_compute(
                    "AllReduce",
                    mybir.AluOpType.add,
                    replica_groups=[list(range(N_CORES))],
                    ins=[ar2_in.opt()],
                    outs=[ar2_out.opt()],
                )
            else:
                nc.gpsimd.dma_start(out=ar2_out, in_=ar2_in)
            g2 = stats.tile([C, 4], F32)
            nc.gpsimd.dma_start(out=g2, in_=ar2_out)

            a2 = stats.tile([C, 2], F32)
            c2 = stats.tile([C, 2], F32)
            gm2 = stats.tile([C, 1], F32)
            gvar2 = stats.tile([C, 1], F32)
            tmp2 = stats.tile([C, 1], F32)
            for hf in range(2):
                nc.scalar.mul(out=gm2, in_=g2[:, 2 * hf : 2 * hf + 1], mul=1.0 / N_CORES)
                nc.scalar.mul(
                    out=gvar2, in_=g2[:, 2 * hf + 1 : 2 * hf + 2], mul=1.0 / N_CORES
                )
                nc.vector.tensor_mul(out=tmp2, in0=gm2, in1=gm2)
                nc.vector.tensor_sub(out=gvar2, in0=gvar2, in1=tmp2)
                nc.scalar.activation(
                    out=gvar2, in_=gvar2,
                    func=mybir.ActivationFunctionType.Sqrt,
                    bias=eps_sb, scale=1.0,
                )
                nc.vector.reciprocal(out=tmp2, in_=gvar2)
                nc.vector.tensor_mul(
                    out=a2[:, hf : hf + 1], in0=tmp2, in1=bn2gb_sb[:, hf : hf + 1]
                )
                nc.vector.tensor_mul(out=tmp2, in0=gm2, in1=a2[:, hf : hf + 1])
                nc.vector.tensor_sub(
                    out=c2[:, hf : hf + 1],
                    in0=bn2gb_sb[:, 2 + hf : 3 + hf], in1=tmp2,
                )

            # ---- stage C: pw pass 2 + fused BN2+ReLU eviction + store -----
            # Two pixel-chunks per half are staged into one [C, 2*PW_CHUNK]
            # tile and shipped with a single 512KB DMA (fewer, bigger DMAs).
            alt = 0
            for n in range(IMG_PER_CORE):
                for pblk in range(0, PIX_PER_IMG, 2 * PW_CHUNK):
                    blk_sz = min(2 * PW_CHUNK, PIX_PER_IMG - pblk)
                    for hf in range(2):
                        ot = yout.tile([C, 2 * PW_CHUNK], F32, tag="yo")
                        for sub in range(0, blk_sz, PW_CHUNK):
                            p0 = pblk + sub
                            sz = min(PW_CHUNK, PIX_PER_IMG - p0)
                            off = n * PIX_PER_IMG + p0
                            hsl = t_sb[:, off : off + sz]
                            # Alternate between the "ps" slots and the (post
                            # stage-B idle) "ps2" slots: 6 PSUM chunks in
                            # flight hide more of the AR2 latency.
                            alt += 1
                            if alt % 3 == 0:
                                py_wide = psum.tile(
                                    [C, 2, PW_CHUNK], F32, tag="ps2", bufs=2,
                                    name=f"pyw{alt}",
                                )
                                py = py_wide[:, 0, :]
                            else:
                                py = psum.tile(
                                    [C, PW_CHUNK], F32, tag="ps", name=f"pyc{alt}"
                                )
                            nc.tensor.matmul(
                                py[:, :sz],
                                pwt_sb[:, hf * 128 : (hf + 1) * 128],
                                hsl.bitcast(F32R),
                                start=True, stop=True,
                            )
                            nc.scalar.activation(
                                out=ot[:, sub : sub + sz], in_=py[:, :sz],
                                func=mybir.ActivationFunctionType.Relu,
                                bias=c2[:, hf : hf + 1],
                                scale=a2[:, hf : hf + 1],
                            )
                        nc.sync.dma_start(
                            out=y_r[n, hf * 128 : (hf + 1) * 128, pblk : pblk + blk_sz],
                            in_=ot[:, :blk_sz],
                        )
    _legalize_waits(nc)
    return nc


_NC_CACHE = []


def prepare(x, dw_w, dw_b, pw_w, pw_b, bn1_g, bn1_b, bn2_g, bn2_b, stride=1, **_):
    # dw_b / pw_b are absorbed by training-mode BN (they only shift the mean,
    # which BN subtracts) and are deliberately unused.
    x = np.asarray(x, dtype=np.float32)
    N = x.shape[0]
    assert x.shape == (16, C, H, W) and N == N_CORES * IMG_PER_CORE

    xp_full = np.zeros((N, C, HP, WP), dtype=np.float32)
    xp_full[:, :, 1 : 1 + H, 1 : 1 + W] = x

    dw9 = np.asarray(dw_w, dtype=np.float32).reshape(C, 9)
    dwdiag = np.zeros((C, 9, C), dtype=np.float32)
    idx = np.arange(C)
    for t in range(9):
        dwdiag[idx, t, idx] = dw9[:, t]

    pwt = np.asarray(pw_w, dtype=np.float32).reshape(O, C).T
    g1 = np.asarray(bn1_g, np.float32)
    b1 = np.asarray(bn1_b, np.float32)
    g2 = np.asarray(bn2_g, np.float32)
    b2 = np.asarray(bn2_b, np.float32)
    cst = np.concatenate(
        [
            dwdiag.reshape(C, 9 * C),
            pwt,
            g1[:, None], b1[:, None],
            g2[:128, None], g2[128:, None], b2[:128, None], b2[128:, None],
            dw9,
        ],
        axis=1,
    ).astype(np.float32)

    if not _NC_CACHE:
        _NC_CACHE.append(_build_program())
    nc = _NC_CACHE[0]

    in_maps = []
    for k in range(N_CORES):
        in_maps.append(
            {
                "xp": np.ascontiguousarray(xp_full[IMG_PER_CORE * k : IMG_PER_CORE * (k + 1)]),
                "cst": cst,
            }
        )

    return nc, in_maps


def kernel(**inputs):
    nc, in_maps = prepare(**inputs)
    res = bass_utils.run_bass_kernel_spmd(
        nc, in_maps, core_ids=list(range(N_CORES))
    )
    out = np.concatenate([r["y"] for r in res.results], axis=0)
    return out


# revision 70
# speedup vs baseline: 333.0794x; 1.0430x over previous
"""Depthwise-separable conv block (dw3x3 + BN + ReLU + pw1x1 + BN + ReLU)
for Trainium2, data-parallel over batch across 8 NeuronCores with sync-BN
via two tiny AllReduces.

Key design points:
  - Depthwise conv = 9 PSUM-accumulated diagonal matmuls per pixel chunk
    (float32r => full PE rate at N>=256).
  - BN in training mode absorbs the conv biases (dw_b, pw_b shift the mean
    which BN subtracts), so they are dropped entirely.
  - BN1 folds to per-channel affine applied by one ScalarE op
    h = relu(a1*t + c1); BN2 folds into the PSUM eviction of the pointwise
    matmul: out = relu(a2*y + c2).
  - BN2 stats need a first pointwise pass (discarded into bn_stats); the
    final pass recomputes y after AllReduce-2 (PE is otherwise idle then).
"""

import numpy as np

import concourse.bass as bass
import concourse.tile as tile
import concourse.mybir as mybir
from concourse import bass_utils

N_CORES = 8
C = 128          # input channels (= SBUF partitions)
O = 256          # output channels
H = W = 112
HP = WP = 114    # zero-padded input
IMG_PER_CORE = 2
PIX_PER_IMG = H * W                 # 12544
PIX_TOTAL = IMG_PER_CORE * PIX_PER_IMG  # 25088
EPS = 1e-5

F32 = mybir.dt.float32
F32R = mybir.dt.float32r

DMA_ROWS = 16    # output rows per input DMA chunk (loads DMA_ROWS+2 rows)
SUB_ROWS = 4     # output rows per conv matmul chunk (N = 448)
PW_CHUNK = 512   # pixels per pointwise matmul (one PSUM bank of f32)

def _legalize_waits(nc):
    """Split multi-wait instructions: this walrus build's codegen accepts at
    most ONE sync wait per ISA instruction, while Tile's sem-assignment
    freely attaches several. Move all but one semaphore wait onto freshly
    inserted NoOps on the same engine directly before the instruction
    (waits are AND-semantics, so order is irrelevant)."""
    cnt = 0
    for bb in nc.main_func.blocks:
        new = []
        for ins in bb.instructions:
            si = ins.sync_info
            if si is not None and len(si.on_wait) > 1:
                sem_waits = [w for w in si.on_wait if w.sync_type == "semaphore"]
                other = [w for w in si.on_wait if w.sync_type != "semaphore"]
                keep = other + sem_waits[-1:] if not other else other
                move = sem_waits[:-1] if not other else sem_waits
                if len(keep) <= 1 and move:
                    for w in move:
                        cnt += 1
                        nop = mybir.InstNoOp(name=f"I-waitnop{cnt}", ins=[], outs=[])
                        nop.engine = ins.engine
                        nop.sync_info = mybir.SyncInfo(on_wait=[w], on_update=[])
                        new.append(nop)
                    ins.sync_info = mybir.SyncInfo(
                        on_wait=keep, on_update=list(si.on_update)
                    )
            new.append(ins)
        try:
            bb.instructions[:] = new
        except TypeError:
            bb.instructions = new
    return cnt


def _build_program(collectives=True, repeat=1):
    nc = bass.Bass(
        "TRN2",
        target_bir_lowering=False,
        debug=False,
        num_devices=N_CORES if collectives else 1,
    )

    # float32r = same 4-byte layout as f32 but lets the PE run matmuls at
    # full rate (fp32 proper is 4 cycles/row); the BIR verifier requires the
    # whole producer chain of a matmul operand to carry the f32r dtype.
    #
    # All constants are packed into ONE tensor so they arrive via one DMA on
    # one DMA lane: Matmult instructions only support a single sync wait, so
    # the first matmul cannot wait on separate weight+data DMA lanes.
    # Layout per channel row:
    #   [dwdiag 9*128 | pwT 256 | g1 b1 | g2h0 g2h1 b2h0 b2h1 | dw9 9]
    NCONST = 9 * C + O + 2 + 4 + 9
    xp = nc.dram_tensor("xp", (IMG_PER_CORE, C, HP, WP), F32R, kind="ExternalInput").ap()
    cst = nc.dram_tensor("cst", (C, NCONST), F32R, kind="ExternalInput").ap()
    y = nc.dram_tensor("y", (IMG_PER_CORE, O, H, W), F32, kind="ExternalOutput").ap()
    y_r = y.rearrange("n c h w -> n c (h w)")

    n_conv_chunks = IMG_PER_CORE * (H // SUB_ROWS)              # 56
    n_pw_chunks = IMG_PER_CORE * ((PIX_PER_IMG + PW_CHUNK - 1) // PW_CHUNK)  # 50

    # `repeat` re-emits the whole body (benchmarking aid: the wall-time slope
    # between repeat=1 and repeat=K programs isolates the on-device time from
    # the ~80ms axon dispatch overhead).
    with tile.TileContext(nc) as tc:
      for _rep in range(repeat):
        with (
            tc.tile_pool(name="consts", bufs=1) as consts,
            tc.tile_pool(name="big", bufs=1) as big,
            tc.tile_pool(name="xin", bufs=4) as xin,
            tc.tile_pool(name="stats", bufs=1) as stats,
            tc.tile_pool(name="yout", bufs=6) as yout,
            # PSUM budget (8 banks): "ps" 4 bufs x 1 bank + "ps2" 2 bufs x 2
            tc.tile_pool(name="psum", bufs=4, space="PSUM") as psum,
            tc.tile_pool(name="dram", bufs=1, space="DRAM") as dram,
        ):
            # ---- constants (single DMA; see NCONST layout note above) -----
            cst_sb = consts.tile([C, NCONST], F32R)
            nc.gpsimd.dma_start(out=cst_sb, in_=cst)
            dwdiag_sb = cst_sb[:, 0 : 9 * C].rearrange("p (t c) -> p t c", t=9)
            pwt_sb = cst_sb[:, 9 * C : 9 * C + O]
            bn1gb_sb = cst_sb[:, 9 * C + O : 9 * C + O + 2].bitcast(F32)
            bn2gb_sb = cst_sb[:, 9 * C + O + 2 : 9 * C + O + 6].bitcast(F32)
            dw9_sb = cst_sb[:, 9 * C + O + 6 : 9 * C + O + 15].bitcast(F32)
            eps_sb = consts.tile([C, 1], F32)
            nc.vector.memset(eps_sb, EPS)

            # depthwise-conv output, SBUF-resident for the whole kernel
            t_sb = big.tile([C, PIX_TOTAL], F32)

            stats1 = stats.tile([C, n_conv_chunks, 6], F32)
            # Stage-B stats: half 0 is fully scanned by DVE bn_stats; half 1
            # is DVE for the first ACT_FROM chunks and ACT (Square+accum_out,
            # one op) afterwards — by then ACT has drained its h-applies.
            # Half-1's mean comes from h row-sums via a tiny matvec instead.
            ACT_FROM = 34
            stats2 = stats.tile([C, n_pw_chunks, 2, 6], F32)
            hsum = stats.tile([C, n_pw_chunks], F32)
            asum = stats.tile([C, n_pw_chunks], F32)
            asq_scratch = stats.tile([C, PW_CHUNK], F32)

            # ---- stage A: depthwise conv + BN1 partial stats --------------
            ci = 0
            for n in range(IMG_PER_CORE):
                for rblk in range(0, H, DMA_ROWS):
                    x_t = xin.tile([C, DMA_ROWS + 2, WP], F32R, tag="x")
                    # HWDGE (nc.sync): RTL descriptor generation, keeps the
                    # Pool engine free (SWDGE costs ~1us of Pool per DMA).
                    # The very first block is split so the first conv matmuls
                    # start after ~390KB instead of a full 1.05MB transfer.
                    if n == 0 and rblk == 0:
                        nc.sync.dma_start(
                            out=x_t[:, 0 : SUB_ROWS + 2, :],
                            in_=xp[n, :, 0 : SUB_ROWS + 2, :],
                        )
                        nc.sync.dma_start(
                            out=x_t[:, SUB_ROWS + 2 :, :],
                            in_=xp[n, :, SUB_ROWS + 2 : DMA_ROWS + 2, :],
                        )
                    else:
                        nc.sync.dma_start(
                            out=x_t, in_=xp[n, :, rblk : rblk + DMA_ROWS + 2, :]
                        )
                    for sr in range(0, DMA_ROWS, SUB_ROWS):
                        pt = psum.tile([C, SUB_ROWS * W], F32, tag="ps")
                        # Taps 0..6 on PE (diagonal matmuls into PSUM); the
                        # remaining taps are scalar_tensor_tensor FMAs on DVE
                        # after the eviction. Tap 7 alternates PE/DVE per
                        # chunk to balance the two engines (GPSIMD lacks
                        # TensorScalarPtr on this target).
                        n_pe_taps = 8 if ci % 3 == 0 else 7
                        for t9 in range(n_pe_taps):
                            di, dj = divmod(t9, 3)
                            rhs = x_t[:, sr + di : sr + di + SUB_ROWS, dj : dj + W]
                            nc.tensor.matmul(
                                pt,
                                dwdiag_sb[:, t9, :],
                                rhs,
                                start=(t9 == 0),
                                stop=(t9 == n_pe_taps - 1),
                            )
                        off = n * PIX_PER_IMG + (rblk + sr) * W
                        tsl = t_sb[:, off : off + SUB_ROWS * W]
                        tsl3 = tsl.rearrange("p (r w) -> p r w", r=SUB_ROWS)
                        nc.scalar.copy(out=tsl.bitcast(F32R), in_=pt)
                        for t9 in range(n_pe_taps, 9):
                            di, dj = divmod(t9, 3)
                            xs = x_t[
                                :, sr + di : sr + di + SUB_ROWS, dj : dj + W
                            ].bitcast(F32)
                            nc.vector.scalar_tensor_tensor(
                                out=tsl3.bitcast(F32R),
                                in0=xs,
                                scalar=dw9_sb[:, t9 : t9 + 1],
                                in1=tsl3,
                                op0=mybir.AluOpType.mult,
                                op1=mybir.AluOpType.add,
                            )
                        nc.vector.bn_stats(out=stats1[:, ci : ci + 1, :], in_=tsl)
                        ci += 1

            # ---- BN1: aggregate, AllReduce, fold --------------------------
            mv1 = stats.tile([C, 2], F32)
            nc.vector.bn_aggr(out=mv1, in_=stats1)
            ar1_sb = stats.tile([C, 2], F32)
            msq1 = stats.tile([C, 1], F32)
            nc.vector.tensor_copy(out=ar1_sb[:, 0:1], in_=mv1[:, 0:1])
            nc.vector.tensor_mul(out=msq1, in0=mv1[:, 0:1], in1=mv1[:, 0:1])
            nc.vector.tensor_add(out=ar1_sb[:, 1:2], in0=mv1[:, 1:2], in1=msq1)

            ar1_in = dram.tile([C, 2], F32)
            ar1_out = dram.tile([C, 2], F32)
            nc.gpsimd.dma_start(out=ar1_in, in_=ar1_sb)
            if collectives:
                nc.gpsimd.collective_compute(
                    "AllReduce",
                    mybir.AluOpType.add,
                    replica_groups=[list(range(N_CORES))],
                    ins=[ar1_in.opt()],
                    outs=[ar1_out.opt()],
                )
            else:  # timing-sim variant: skip the collective
                nc.gpsimd.dma_start(out=ar1_out, in_=ar1_in)
            g1 = stats.tile([C, 2], F32)
            nc.gpsimd.dma_start(out=g1, in_=ar1_out)

            gm1 = stats.tile([C, 1], F32)
            gex1 = stats.tile([C, 1], F32)
            gvar1 = stats.tile([C, 1], F32)
            a1 = stats.tile([C, 1], F32)
            c1 = stats.tile([C, 1], F32)
            nc.scalar.mul(out=gm1, in_=g1[:, 0:1], mul=1.0 / N_CORES)
            nc.scalar.mul(out=gex1, in_=g1[:, 1:2], mul=1.0 / N_CORES)
            nc.vector.tensor_mul(out=gvar1, in0=gm1, in1=gm1)
            nc.vector.tensor_sub(out=gvar1, in0=gex1, in1=gvar1)
            nc.scalar.activation(
                out=gvar1, in_=gvar1,
                func=mybir.ActivationFunctionType.Sqrt,
                bias=eps_sb, scale=1.0,
            )
            nc.vector.reciprocal(out=a1, in_=gvar1)
            nc.vector.tensor_mul(out=a1, in0=a1, in1=bn1gb_sb[:, 0:1])
            nc.vector.tensor_mul(out=c1, in0=gm1, in1=a1)
            nc.vector.tensor_sub(out=c1, in0=bn1gb_sb[:, 1:2], in1=c1)

            # ---- stage B: h = relu(a1*t + c1); pw pass 1 -> BN2 stats -----
            # All h-applies are issued first: they depend only on AR1, so ACT
            # streams through them without waiting on PE (interleaving them
            # with the PE-dependent stats ops would serialize ACT's FIFO on
            # per-chunk round-trips).
            chunks = []
            for n in range(IMG_PER_CORE):
                for p0 in range(0, PIX_PER_IMG, PW_CHUNK):
                    sz = min(PW_CHUNK, PIX_PER_IMG - p0)
                    off = n * PIX_PER_IMG + p0
                    cj = len(chunks)
                    chunks.append((off, sz))
                    hsl = t_sb[:, off : off + sz]
                    nc.scalar.activation(
                        out=hsl.bitcast(F32R), in_=hsl,
                        func=mybir.ActivationFunctionType.Relu,
                        bias=c1, scale=a1,
                        accum_out=hsum[:, cj : cj + 1],
                    )

            for cj, (off, sz) in enumerate(chunks):
                hsl = t_sb[:, off : off + sz]
                py2 = psum.tile([C, 2, PW_CHUNK], F32, tag="ps2", bufs=2)
                for hf in range(2):
                    nc.tensor.matmul(
                        py2[:, hf, :sz],
                        pwt_sb[:, hf * 128 : (hf + 1) * 128],
                        hsl.bitcast(F32R),
                        start=True, stop=True,
                    )
                nc.vector.bn_stats(
                    out=stats2[:, cj : cj + 1, 0, :], in_=py2[:, 0, :sz]
                )
                if cj < ACT_FROM:
                    nc.vector.bn_stats(
                        out=stats2[:, cj : cj + 1, 1, :], in_=py2[:, 1, :sz]
                    )
                else:
                    nc.scalar.activation(
                        out=asq_scratch[:, :sz], in_=py2[:, 1, :sz],
                        func=mybir.ActivationFunctionType.Square,
                        accum_out=asum[:, cj : cj + 1],
                    )

            # ---- BN2: aggregate, AllReduce, fold --------------------------
            n_tot = float(PIX_TOTAL)
            n_d1 = float(sum(sz for j, (_, sz) in enumerate(chunks) if j < ACT_FROM))
            mv2 = stats.tile([C, 2, 2], F32)
            ar2_sb = stats.tile([C, 4], F32)
            msq2 = stats.tile([C, 1], F32)
            part2 = stats.tile([C, 1], F32)
            hs = stats.tile([C, 1], F32)

            # half 0: fully DVE-scanned
            nc.vector.bn_aggr(out=mv2[:, 0, :], in_=stats2[:, :, 0, :])
            nc.vector.tensor_copy(out=ar2_sb[:, 0:1], in_=mv2[:, 0, 0:1])
            nc.vector.tensor_mul(out=msq2, in0=mv2[:, 0, 0:1], in1=mv2[:, 0, 0:1])
            nc.vector.tensor_add(out=ar2_sb[:, 1:2], in0=mv2[:, 0, 1:2], in1=msq2)

            # half 1 mean: sum_p y1[o,p] = pwT_half1^T @ (sum_p h[:,p]).
            # The matvec is padded to N=8 (fp32r matmuls reject N=1).
            hs8 = stats.tile([C, 8], F32R)
            nc.vector.tensor_scalar_mul(out=hs8, in0=hsum[:, 0:8], scalar1=0.0)
            with nc.allow_low_precision(reason="f32r is f32 bits; matmul needs the dtype"):
                nc.vector.reduce_sum(
                    out=hs8[:, 0:1], in_=hsum, axis=mybir.AxisListType.X
                )
            pmv = psum.tile([C, 8], F32, tag="ps")
            nc.tensor.matmul(
                pmv, pwt_sb[:, 128:256], hs8, start=True, stop=True
            )
            nc.vector.tensor_scalar_mul(
                out=ar2_sb[:, 2:3], in0=pmv[:, 0:1], scalar1=1.0 / n_tot
            )
            # half 1 E[y^2]: DVE-subset moments + ACT sumsq tail
            nc.vector.bn_aggr(out=mv2[:, 1, :], in_=stats2[:, :ACT_FROM, 1, :])
            nc.vector.tensor_mul(out=msq2, in0=mv2[:, 1, 0:1], in1=mv2[:, 1, 0:1])
            nc.vector.tensor_add(out=msq2, in0=mv2[:, 1, 1:2], in1=msq2)
            nc.vector.reduce_sum(
                out=part2, in_=asum[:, ACT_FROM:], axis=mybir.AxisListType.X
            )
            nc.vector.tensor_scalar_mul(out=part2, in0=part2, scalar1=1.0 / n_tot)
            nc.vector.scalar_tensor_tensor(
                out=ar2_sb[:, 3:4],
                in0=msq2, scalar=n_d1 / n_tot, in1=part2,
                op0=mybir.AluOpType.mult, op1=mybir.AluOpType.add,
            )

            ar2_in = dram.tile([C, 4], F32)
            ar2_out = dram.tile([C, 4], F32)
            nc.gpsimd.dma_start(out=ar2_in, in_=ar2_sb)
            if collectives:
                nc.gpsimd.collective_compute(
                    "AllReduce",
                    mybir.AluOpType.add,
                    replica_groups=[list(range(N_CORES))],
                    ins=[ar2_in.opt()],
                    outs=[ar2_out.opt()],
                )
            else:
                nc.gpsimd.dma_start(out=ar2_out, in_=ar2_in)
            g2 = stats.tile([C, 4], F32)
            nc.gpsimd.dma_start(out=g2, in_=ar2_out)

            a2 = stats.tile([C, 2], F32)
            c2 = stats.tile([C, 2], F32)
            gm2 = stats.tile([C, 1], F32)
            gvar2 = stats.tile([C, 1], F32)
            tmp2 = stats.tile([C, 1], F32)
            for hf in range(2):
                nc.scalar.mul(out=gm2, in_=g2[:, 2 * hf : 2 * hf + 1], mul=1.0 / N_CORES)
                nc.scalar.mul(
                    out=gvar2, in_=g2[:, 2 * hf + 1 : 2 * hf + 2], mul=1.0 / N_CORES
                )
                nc.vector.tensor_mul(out=tmp2, in0=gm2, in1=gm2)
                nc.vector.tensor_sub(out=gvar2, in0=gvar2, in1=tmp2)
                nc.scalar.activation(
                    out=gvar2, in_=gvar2,
                    func=mybir.ActivationFunctionType.Sqrt,
                    bias=eps_sb, scale=1.0,
                )
                nc.vector.reciprocal(out=tmp2, in_=gvar2)
                nc.vector.tensor_mul(
                    out=a2[:, hf : hf + 1], in0=tmp2, in1=bn2gb_sb[:, hf : hf + 1]
                )
                nc.vector.tensor_mul(out=tmp2, in0=gm2, in1=a2[:, hf : hf + 1])
                nc.vector.tensor_sub(
                    out=c2[:, hf : hf + 1],
                    in0=bn2gb_sb[:, 2 + hf : 3 + hf], in1=tmp2,
                )

            # ---- stage C: pw pass 2 + fused BN2+ReLU eviction + store -----
            # Two pixel-chunks per half are staged into one [C, 2*PW_CHUNK]
            # tile and shipped with a single 512KB DMA (fewer, bigger DMAs).
            alt = 0
            for n in range(IMG_PER_CORE):
                for pblk in range(0, PIX_PER_IMG, 2 * PW_CHUNK):
                    blk_sz = min(2 * PW_CHUNK, PIX_PER_IMG - pblk)
                    for hf in range(2):
                        ot = yout.tile([C, 2 * PW_CHUNK], F32, tag="yo")
                        for sub in range(0, blk_sz, PW_CHUNK):
                            p0 = pblk + sub
                            sz = min(PW_CHUNK, PIX_PER_IMG - p0)
                            off = n * PIX_PER_IMG + p0
                            hsl = t_sb[:, off : off + sz]
                            # Alternate between the "ps" slots and the (post
                            # stage-B idle) "ps2" slots: 6 PSUM chunks in
                            # flight hide more of the AR2 latency.
                            alt += 1
                            if alt % 3 == 0:
                                py_wide = psum.tile(
                                    [C, 2, PW_CHUNK], F32, tag="ps2", bufs=2,
                                    name=f"pyw{alt}",
                                )
                                py = py_wide[:, 0, :]
                            else:
                                py = psum.tile(
                                    [C, PW_CHUNK], F32, tag="ps", name=f"pyc{alt}"
                                )
                            nc.tensor.matmul(
                                py[:, :sz],
                                pwt_sb[:, hf * 128 : (hf + 1) * 128],
                                hsl.bitcast(F32R),
                                start=True, stop=True,
                            )
                            nc.scalar.activation(
                                out=ot[:, sub : sub + sz], in_=py[:, :sz],
                                func=mybir.ActivationFunctionType.Relu,
                                bias=c2[:, hf : hf + 1],
                                scale=a2[:, hf : hf + 1],
                            )
                        nc.sync.dma_start(
                            out=y_r[n, hf * 128 : (hf + 1) * 128, pblk : pblk + blk_sz],
                            in_=ot[:, :blk_sz],
                        )
    _legalize_waits(nc)
    return nc


_NC_CACHE = []


def prepare(x, dw_w, dw_b, pw_w, pw_b, bn1_g, bn1_b, bn2_g, bn2_b, stride=1, **_):
    # dw_b / pw_b are absorbed by training-mode BN (they only shift the mean,
    # which BN subtracts) and are deliberately unused.
    x = np.asarray(x, dtype=np.float32)
    N = x.shape[0]
    assert x.shape == (16, C, H, W) and N == N_CORES * IMG_PER_CORE

    xp_full = np.zeros((N, C, HP, WP), dtype=np.float32)
    xp_full[:, :, 1 : 1 + H, 1 : 1 + W] = x

    dw9 = np.asarray(dw_w, dtype=np.float32).reshape(C, 9)
    dwdiag = np.zeros((C, 9, C), dtype=np.float32)
    idx = np.arange(C)
    for t in range(9):
        dwdiag[idx, t, idx] = dw9[:, t]

    pwt = np.asarray(pw_w, dtype=np.float32).reshape(O, C).T
    g1 = np.asarray(bn1_g, np.float32)
    b1 = np.asarray(bn1_b, np.float32)
    g2 = np.asarray(bn2_g, np.float32)
    b2 = np.asarray(bn2_b, np.float32)
    cst = np.concatenate(
        [
            dwdiag.reshape(C, 9 * C),
            pwt,
            g1[:, None], b1[:, None],
            g2[:128, None], g2[128:, None], b2[:128, None], b2[128:, None],
            dw9,
        ],
        axis=1,
    ).astype(np.float32)

    if not _NC_CACHE:
        _NC_CACHE.append(_build_program())
    nc = _NC_CACHE[0]

    in_maps = []
    for k in range(N_CORES):
        in_maps.append(
            {
                "xp": np.ascontiguousarray(xp_full[IMG_PER_CORE * k : IMG_PER_CORE * (k + 1)]),
                "cst": cst,
            }
        )

    return nc, in_maps


def kernel(**inputs):
    nc, in_maps = prepare(**inputs)
    res = bass_utils.run_bass_kernel_spmd(
        nc, in_maps, core_ids=list(range(N_CORES))
    )
    out = np.concatenate([r["y"] for r in res.results], axis=0)
    return out
